# revision 1
# baseline (speedup 1.0000x reference)
"""Trainium2 Bass kernel for nn_KnowledgeAttention.

Math (per batch example b):
    sim[k]  = cos_sim(pooled[b], kg_key[b,k])                      # [K]
    q       = (hs @ Wq.T + bq) * HD**-0.5     -> heads [T,H,HD]
    k       = kg_value @ Wk.T + bk            -> heads [K,H,HD]
    v       = kg_value @ Wv.T + bv            -> heads [K,H,HD]
    S[h,t,k]= q_h[t]·k_h[k] + beta[h]*sim[k]
    P       = softmax_k(S);  O[t,h] = sum_k P v
    out     = O @ Wo.T + bo

Sharding: pure data-parallel over batch — 8 examples on 8 cores, weights
replicated, no collectives.

Per-core layout strategy (all matmul contractions run on the partition dim):
    hs.T, kg_value.T via PE transpose; q.T/k.T/v from projections;
    scores computed transposed S.T[k,t] so the cosine-sim bias is a
    per-partition scalar folded into the ACT exp bias; attention output
    O.T[d,t] feeds the final projection lhsT directly; softmax denominators
    via ones-matmuls; normalization uses a gpsimd partition-broadcast of the
    reciprocal row. Matmuls in bf16 with fp32 PSUM accumulation.
"""

import sys

import numpy as np

# ---------------------------------------------------------------- constants
BS = 8
T = 2048
D = 768
H = 12
HD = 64
K = 512
SCALE = HD ** -0.5
EPS = 1e-8
DC = D // 128   # 6 contraction/partition chunks of 128 over D
KC = K // 128   # 4 chunks over K
TW = 512        # t window for moving operand
NTW = T // TW   # 4
NPAIR = H // 2  # 6 head pairs

TRACE = False
LAST_EXEC_NS = None

_CACHE = {}


def _ensure_path():
    try:
        import concourse  # noqa: F401
    except ImportError:
        for p in ("/opt/trn_rl_repo", "/root/.axon_site/_ro/trn_rl_repo"):
            if p not in sys.path:
                sys.path.insert(0, p)


def _build_program():
    _ensure_path()
    import concourse.bass as bass
    import concourse.mybir as mybir
    import concourse.tile as tile
    from concourse import bacc
    from concourse.masks import make_identity
    from contextlib import ExitStack

    F32 = mybir.dt.float32
    BF16 = mybir.dt.bfloat16
    Alu = mybir.AluOpType
    Act = mybir.ActivationFunctionType

    nc = bacc.Bacc("TRN2", target_bir_lowering=False, debug=False, num_devices=BS)

    hs_d = nc.dram_tensor("hs", [T, D], F32, kind="ExternalInput").ap()
    kgk_d = nc.dram_tensor("kgk", [K, D], F32, kind="ExternalInput").ap()
    kgv_d = nc.dram_tensor("kgv", [K, D], F32, kind="ExternalInput").ap()
    pl_d = nc.dram_tensor("pooled", [1, D], F32, kind="ExternalInput").ap()
    wqt_d = nc.dram_tensor("wqt", [D, D], BF16, kind="ExternalInput").ap()
    wkt_d = nc.dram_tensor("wkt", [D, D], BF16, kind="ExternalInput").ap()
    wvt_d = nc.dram_tensor("wvt", [D, D], BF16, kind="ExternalInput").ap()
    wot_d = nc.dram_tensor("wot", [D, D], BF16, kind="ExternalInput").ap()
    bq_d = nc.dram_tensor("bq", [128, DC], F32, kind="ExternalInput").ap()
    bk_d = nc.dram_tensor("bk", [128, DC], F32, kind="ExternalInput").ap()
    bo_d = nc.dram_tensor("bo", [1, D], F32, kind="ExternalInput").ap()
    beta_d = nc.dram_tensor("beta", [1, H], F32, kind="ExternalInput").ap()
    out_d = nc.dram_tensor("out", [T, D], F32, kind="ExternalOutput").ap()

    with tile.TileContext(nc) as tc, ExitStack() as ctx:
        const = ctx.enter_context(tc.tile_pool(name="const", bufs=1))
        inp = ctx.enter_context(tc.tile_pool(name="inp", bufs=6))
        wpool = ctx.enter_context(tc.tile_pool(name="w", bufs=12))
        big = ctx.enter_context(tc.tile_pool(name="big", bufs=12))
        hstw_p = ctx.enter_context(tc.tile_pool(name="hstw", bufs=12))
        kt_p = ctx.enter_context(tc.tile_pool(name="ktp", bufs=6))
        v_p = ctx.enter_context(tc.tile_pool(name="vp", bufs=4))
        kgt_p = ctx.enter_context(tc.tile_pool(name="kgtp", bufs=6))
        e_p = ctx.enter_context(tc.tile_pool(name="ep", bufs=12))
        r_p = ctx.enter_context(tc.tile_pool(name="rp", bufs=4))
        rb_p = ctx.enter_context(tc.tile_pool(name="rbp", bufs=2))
        fin_p = ctx.enter_context(tc.tile_pool(name="finp", bufs=2))
        sm_p = ctx.enter_context(tc.tile_pool(name="smp", bufs=4))
        ps = ctx.enter_context(tc.tile_pool(name="ps", bufs=2, space="PSUM"))

        # ---------------- phase 0: constants + cosine-sim bias ----------------
        ident = const.tile([128, 128], F32, tag="ident")
        make_identity(nc, ident[:])
        ones_bf = const.tile([128, 32], BF16, tag="ones_bf")
        nc.vector.memset(ones_bf[:], 1.0)
        # kg_value loads first: transposes are the critical path into phase 1
        kv_tiles = []
        for c in range(KC):
            kv = inp.tile([128, D], F32, tag="inp", name="kv")
            nc.sync.dma_start(kv[:], kgv_d[c * 128:(c + 1) * 128, :])
            kv_tiles.append(kv)

        pl = const.tile([1, D], F32, tag="pl")
        nc.sync.dma_start(pl[:], pl_d)
        bt = const.tile([1, H], F32, tag="bt")
        nc.sync.dma_start(bt[:], beta_d)
        bo_row = const.tile([1, D], F32, tag="bo_row")
        nc.sync.dma_start(bo_row[:], bo_d)
        bq_sb = const.tile([128, DC], F32, tag="bq_sb")
        nc.sync.dma_start(bq_sb[:], bq_d)
        bk_sb = const.tile([128, DC], F32, tag="bk_sb")
        nc.sync.dma_start(bk_sb[:], bk_d)

        bo_bc = const.tile([128, D], F32, tag="bo_bc")
        nc.gpsimd.partition_broadcast(bo_bc[:], bo_row[:], channels=128)
        beta_bc = const.tile([128, H], F32, tag="beta_bc")
        nc.gpsimd.partition_broadcast(beta_bc[:], bt[:], channels=128)
        pl_bc = const.tile([128, D], F32, tag="pl_bc")
        nc.gpsimd.partition_broadcast(pl_bc[:], pl[:], channels=128)

        # pooled 1/||.|| as a per-partition vector (computed on the broadcast)
        pl_sq = inp.tile([128, D], F32, tag="inp", name="pl_sq")
        pnorm = sm_p.tile([128, 1], F32, tag="pnorm")
        nc.scalar.activation(pl_sq[:], pl_bc[:], Act.Square, accum_out=pnorm[:])
        nc.scalar.activation(pnorm[:], pnorm[:], Act.Sqrt)
        nc.vector.tensor_scalar_max(pnorm[:], pnorm[:], EPS)
        rp_vec = const.tile([128, 1], F32, tag="rp_vec")
        nc.vector.reciprocal(rp_vec[:], pnorm[:])

        # bias_all[k_part, kc*H + h] = beta[h] * sim[k]
        bias_all = const.tile([128, KC * H], F32, tag="bias_all")
        for c in range(KC):
            kk = inp.tile([128, D], F32, tag="inp")
            nc.sync.dma_start(kk[:], kgk_d[c * 128:(c + 1) * 128, :])
            sq = inp.tile([128, D], F32, tag="inp")
            nrm = sm_p.tile([128, 1], F32, tag="nrm")
            nc.scalar.activation(sq[:], kk[:], Act.Square, accum_out=nrm[:])
            nc.scalar.activation(nrm[:], nrm[:], Act.Sqrt)
            nc.vector.tensor_scalar_max(nrm[:], nrm[:], EPS)
            rn = sm_p.tile([128, 1], F32, tag="rn")
            nc.vector.reciprocal(rn[:], nrm[:])
            sq2 = inp.tile([128, D], F32, tag="inp")
            dot = sm_p.tile([128, 1], F32, tag="dot")
            nc.vector.scalar_tensor_tensor(
                out=sq2[:], in0=kk[:], scalar=1.0, in1=pl_bc[:],
                op0=Alu.mult, op1=Alu.mult, accum_out=dot[:])
            nc.vector.tensor_mul(dot[:], dot[:], rn[:])
            nc.vector.tensor_mul(dot[:], dot[:], rp_vec[:])
            nc.vector.tensor_scalar_mul(
                bias_all[:, c * H:(c + 1) * H], beta_bc[:], dot[:])

        # ---------------- phase 1a: kg_value.T, k.T, v ----------------
        wk_sb = []
        wv_sb = []
        for c in range(DC):
            wk = wpool.tile([128, D], BF16, tag="w")
            nc.sync.dma_start(wk[:], wkt_d[c * 128:(c + 1) * 128, :])
            wk_sb.append(wk)
        for c in range(DC):
            wv = wpool.tile([128, D], BF16, tag="w")
            nc.sync.dma_start(wv[:], wvt_d[c * 128:(c + 1) * 128, :])
            wv_sb.append(wv)

        kgt = [kgt_p.tile([128, K], BF16, tag="kgt", name="kgt") for _ in range(DC)]
        for dchunk in range(DC):
            pt = ps.tile([128, K], F32, tag="s", bufs=2, name="ptr")
            for c in range(KC):
                nc.tensor.transpose(
                    pt[:, c * 128:(c + 1) * 128],
                    kv_tiles[c][:, dchunk * 128:(dchunk + 1) * 128], ident[:])
            nc.vector.tensor_copy(kgt[dchunk][:], pt[:])

        kt = [kt_p.tile([128, K], BF16, tag="kt", name="kt") for _ in range(DC)]
        for m in range(DC):
            pk = ps.tile([128, K], F32, tag="mm", bufs=2)
            for c in range(DC):
                nc.tensor.matmul(
                    pk[:], wk_sb[c][:, m * 128:(m + 1) * 128], kgt[c][:],
                    start=(c == 0), stop=(c == DC - 1))
            nc.vector.tensor_scalar_add(kt[m][:], pk[:], bk_sb[:, m:m + 1])

        v_sb = [v_p.tile([128, D], BF16, tag="v", name="vsb")
                for _ in range(KC)]
        for kc in range(KC):
            for n in range(2):
                pv = ps.tile([128, 384], F32, tag="mm", bufs=2)
                for c in range(DC):
                    nc.tensor.matmul(
                        pv[:], kgt[c][:, kc * 128:(kc + 1) * 128],
                        wv_sb[c][:, n * 384:(n + 1) * 384],
                        start=(c == 0), stop=(c == DC - 1))
                nc.vector.tensor_copy(
                    v_sb[kc][:, n * 384:(n + 1) * 384], pv[:])

        # ---------------- phase 1b: hs.T windows + q.T ----------------
        wq_sb = []
        for c in range(DC):
            wq = wpool.tile([128, D], BF16, tag="w")
            nc.sync.dma_start(wq[:], wqt_d[c * 128:(c + 1) * 128, :])
            wq_sb.append(wq)

        qt = [big.tile([128, T], BF16, tag="big", name="qt") for _ in range(DC)]
        for tc4 in range(NTW):
            hstw = [hstw_p.tile([128, TW], BF16, tag="hstw", name="hstw") for _ in range(DC)]
            hv_tiles = []
            for tsub in range(TW // 128):
                hv = inp.tile([128, D], F32, tag="inp")
                t0 = tc4 * TW + tsub * 128
                nc.sync.dma_start(hv[:], hs_d[t0:t0 + 128, :])
                hv_tiles.append(hv)
            for c in range(DC):
                pt = ps.tile([128, TW], F32, tag="s", bufs=2, name="ptr")
                for tsub in range(TW // 128):
                    nc.tensor.transpose(
                        pt[:, tsub * 128:(tsub + 1) * 128],
                        hv_tiles[tsub][:, c * 128:(c + 1) * 128], ident[:])
                nc.vector.tensor_copy(hstw[c][:], pt[:])
            for m in range(DC):
                pq = ps.tile([128, TW], F32, tag="mm", bufs=2)
                for c in range(DC):
                    nc.tensor.matmul(
                        pq[:], wq_sb[c][:, m * 128:(m + 1) * 128], hstw[c][:],
                        start=(c == 0), stop=(c == DC - 1))
                nc.vector.tensor_scalar_add(
                    qt[m][:, tc4 * TW:(tc4 + 1) * TW], pq[:], bq_sb[:, m:m + 1])

        # ------- phase 2+3 interleaved: attention + final proj per t-window -------
        wo_sb = []
        for c in range(DC):
            wo = wpool.tile([128, D], BF16, tag="w")
            nc.sync.dma_start(wo[:], wot_d[c * 128:(c + 1) * 128, :])
            wo_sb.append(wo)

        ot = [big.tile([128, T], BF16, tag="big", name="ot") for _ in range(NPAIR)]
        for tc4 in range(NTW):
            tw = slice(tc4 * TW, (tc4 + 1) * TW)
            for g in range(NPAIR // 2):
                e_all = []          # [jj][kc][even/odd]
                for jj in range(2):
                    j = 2 * g + jj
                    e_j = []
                    for kc in range(KC):
                        pse = ps.tile([128, TW], F32, tag="s", bufs=2)
                        nc.tensor.matmul(
                            pse[:], kt[j][0:64, kc * 128:(kc + 1) * 128],
                            qt[j][0:64, tw], start=True, stop=True)
                        pso = ps.tile([128, TW], F32, tag="s", bufs=2)
                        nc.tensor.matmul(
                            pso[:], kt[j][64:128, kc * 128:(kc + 1) * 128],
                            qt[j][64:128, tw], start=True, stop=True)
                        ee = e_p.tile([128, TW], BF16, tag="e")
                        h0 = kc * H + 2 * j
                        nc.scalar.activation(
                            ee[:], pse[:], Act.Exp,
                            bias=bias_all[:, h0:h0 + 1], scale=1.0)
                        eo = e_p.tile([128, TW], BF16, tag="e")
                        nc.scalar.activation(
                            eo[:], pso[:], Act.Exp,
                            bias=bias_all[:, h0 + 1:h0 + 2], scale=1.0)
                        e_j.append((ee, eo))
                    e_all.append(e_j)

                pd = ps.tile([128, TW], F32, tag="d", bufs=2, name="pd")
                po_g = []
                for jj in range(2):
                    j = 2 * g + jj
                    po = ps.tile([128, TW], F32, tag="o", bufs=2, name="po")
                    po_g.append(po)
                    # AV: even head rows 0:64, odd head rows 64:128 (two
                    # sequential col-tiled accumulation chains in one bank)
                    for kc in range(KC):
                        nc.tensor.matmul(
                            po[0:64, :],
                            v_sb[kc][:, (2 * j) * HD:(2 * j + 1) * HD],
                            e_all[jj][kc][0][:],
                            start=(kc == 0), stop=(kc == KC - 1))
                    for kc in range(KC):
                        nc.tensor.matmul(
                            po[64:128, :],
                            v_sb[kc][:, (2 * j + 1) * HD:(2 * j + 2) * HD],
                            e_all[jj][kc][1][:],
                            start=(kc == 0), stop=(kc == KC - 1))
                    # denominators, 32x-replicated into the group's pd bank
                    for kc in range(KC):
                        nc.tensor.matmul(
                            pd[jj * 64:jj * 64 + 32, :], ones_bf[:, 0:32],
                            e_all[jj][kc][0][:],
                            start=(kc == 0), stop=(kc == KC - 1),
                            tile_position=(0, jj * 64))
                    for kc in range(KC):
                        nc.tensor.matmul(
                            pd[jj * 64 + 32:jj * 64 + 64, :], ones_bf[:, 0:32],
                            e_all[jj][kc][1][:],
                            start=(kc == 0), stop=(kc == KC - 1),
                            tile_position=(0, jj * 64 + 32))

                rall = r_p.tile([128, TW], F32, tag="rall", name="rall")
                nc.vector.reciprocal_approx_fast(rall[:], pd[:])
                for jj in range(2):
                    j = 2 * g + jj
                    po = po_g[jj]
                    b = jj * 64
                    nc.vector.tensor_mul(
                        ot[j][0:32, tw], po[0:32, :], rall[b:b + 32, :])
                    nc.vector.tensor_mul(
                        ot[j][32:64, tw], po[32:64, :], rall[b:b + 32, :])
                    nc.vector.tensor_mul(
                        ot[j][64:96, tw], po[64:96, :], rall[b + 32:b + 64, :])
                    nc.vector.tensor_mul(
                        ot[j][96:128, tw], po[96:128, :], rall[b + 32:b + 64, :])

            for tsub in range(TW // 128):
                tc16 = tc4 * (TW // 128) + tsub
                fin = fin_p.tile([128, D], F32, tag="fin")
                for n in range(2):
                    pf = ps.tile([128, 384], F32, tag="mm", bufs=2)
                    for c in range(DC):
                        nc.tensor.matmul(
                            pf[:], ot[c][:, tc16 * 128:(tc16 + 1) * 128],
                            wo_sb[c][:, n * 384:(n + 1) * 384],
                            start=(c == 0), stop=(c == DC - 1))
                    nc.vector.tensor_add(
                        fin[:, n * 384:(n + 1) * 384], pf[:],
                        bo_bc[:, n * 384:(n + 1) * 384])
                nc.sync.dma_start(out_d[tc16 * 128:(tc16 + 1) * 128, :], fin[:])

    nc.compile()
    return nc


def _get_program():
    if "nc" not in _CACHE:
        _CACHE["nc"] = _build_program()
    return _CACHE["nc"]


def _host_prep(inputs):
    import ml_dtypes
    bf16 = ml_dtypes.bfloat16

    f32 = lambda x: np.ascontiguousarray(np.asarray(x, dtype=np.float32))
    Wq, Wk, Wv, Wo = (f32(inputs[k]) for k in ("Wq", "Wk", "Wv", "Wo"))
    bq, bk, bv, bo = (f32(inputs[k]) for k in ("bq", "bk", "bv", "bo"))
    beta = f32(inputs["beta"])

    shared = {
        "wqt": np.ascontiguousarray((Wq.T * SCALE).astype(bf16)),
        "wkt": np.ascontiguousarray(Wk.T.astype(bf16)),
        "wvt": np.ascontiguousarray(Wv.T.astype(bf16)),
        "wot": np.ascontiguousarray(Wo.T.astype(bf16)),
        "bq": np.ascontiguousarray((bq * SCALE).reshape(DC, 128).T),
        "bk": np.ascontiguousarray(bk.reshape(DC, 128).T),
        # bv folded through Wo (sum_k softmax == 1), bo absorbed:
        "bo": np.ascontiguousarray((bo + bv @ Wo.T).reshape(1, D)),
        "beta": np.ascontiguousarray(beta.reshape(1, H)),
    }

    hs = f32(inputs["hidden_states"])
    kgk = f32(inputs["kg_key"])
    kgv = f32(inputs["kg_value"])
    pooled = f32(inputs["pooled_hidden_states"])

    in_maps = []
    for b in range(BS):
        m = dict(shared)
        m["hs"] = np.ascontiguousarray(hs[b])
        m["kgk"] = np.ascontiguousarray(kgk[b])
        m["kgv"] = np.ascontiguousarray(kgv[b])
        m["pooled"] = np.ascontiguousarray(pooled[b].reshape(1, D))
        in_maps.append(m)
    return in_maps




def _install_ntff_hook():
    """Register the axon NTFF profile hook so trace=True yields exec_time_ns.

    Only used from our own test harness (TRACE=True); the default kernel()
    path never calls this.
    """
    try:
        from antenv.axon_hooks import get_axon_ntff_profile_hook  # noqa: F401
        return
    except ImportError:
        pass
    import contextlib
    import ctypes
    import types

    so_path = "/opt/axon/libaxon_pjrt.so"
    try:
        lib = ctypes.CDLL(so_path)
    except OSError:
        return
    if not hasattr(lib, "axon_start_nrt_profile"):
        return
    lib.axon_start_nrt_profile.argtypes = [
        ctypes.POINTER(ctypes.c_int64), ctypes.c_size_t]
    lib.axon_start_nrt_profile.restype = ctypes.c_int64
    lib.axon_stop_nrt_profile.argtypes = [ctypes.c_char_p]
    lib.axon_stop_nrt_profile.restype = ctypes.c_int64

    @contextlib.contextmanager
    def _hook(output_dir, device_ids):
        import jax
        jax.devices()
        if device_ids:
            ids = (ctypes.c_int64 * len(device_ids))(*device_ids)
            rc = lib.axon_start_nrt_profile(ids, len(device_ids))
        else:
            rc = lib.axon_start_nrt_profile(None, 0)
        if rc != 0:
            raise RuntimeError(f"axon_start_nrt_profile rc={rc}")
        try:
            yield
        finally:
            n = lib.axon_stop_nrt_profile(str(output_dir).encode())
            print(f"profile: {n} file(s) written to {output_dir}",
                  file=sys.stderr)

    mod = types.ModuleType("antenv.axon_hooks")
    mod.get_axon_ntff_profile_hook = lambda: _hook
    mod.set_axon_ntff_profile_hook = lambda h: None
    sys.modules["antenv.axon_hooks"] = mod


def kernel(**inputs):
    global LAST_EXEC_NS
    _ensure_path()
    from concourse import bass_utils

    if TRACE:
        _install_ntff_hook()
    nc = _get_program()
    in_maps = _host_prep(inputs)
    res = bass_utils.run_bass_kernel_spmd(
        nc, in_maps, core_ids=list(range(BS)), trace=TRACE)
    LAST_EXEC_NS = res.exec_time_ns
    out = np.stack([res.results[b]["out"] for b in range(BS)], axis=0)
    return out.astype(np.float32)



# revision 7
# speedup vs baseline: 1.0693x; 1.0693x over previous
"""Trainium2 Bass kernel for nn_KnowledgeAttention.

Math (per batch example b):
    sim[k]  = cos_sim(pooled[b], kg_key[b,k])                      # [K]
    q       = (hs @ Wq.T + bq) * HD**-0.5     -> heads [T,H,HD]
    k       = kg_value @ Wk.T + bk            -> heads [K,H,HD]
    v       = kg_value @ Wv.T + bv            -> heads [K,H,HD]
    S[h,t,k]= q_h[t]·k_h[k] + beta[h]*sim[k]
    P       = softmax_k(S);  O[t,h] = sum_k P v
    out     = O @ Wo.T + bo

Sharding: pure data-parallel over batch — 8 examples on 8 cores, weights
replicated, no collectives.

Per-core layout strategy (all matmul contractions run on the partition dim):
    hs.T, kg_value.T via PE transpose; q.T/k.T/v from projections;
    scores computed transposed S.T[k,t] so the cosine-sim bias is a
    per-partition scalar folded into the ACT exp bias; attention output
    O.T[d,t] feeds the final projection lhsT directly; softmax denominators
    via ones-matmuls; normalization uses a gpsimd partition-broadcast of the
    reciprocal row. Matmuls in bf16 with fp32 PSUM accumulation.
"""

import sys

import numpy as np

# ---------------------------------------------------------------- constants
BS = 8
T = 2048
D = 768
H = 12
HD = 64
K = 512
SCALE = HD ** -0.5
EPS = 1e-8
DC = D // 128   # 6 contraction/partition chunks of 128 over D
KC = K // 128   # 4 chunks over K
TW = 512        # t window for moving operand
NTW = T // TW   # 4
NPAIR = H // 2  # 6 head pairs

TRACE = False
LAST_EXEC_NS = None

_CACHE = {}


def _ensure_path():
    try:
        import concourse  # noqa: F401
    except ImportError:
        for p in ("/opt/trn_rl_repo", "/root/.axon_site/_ro/trn_rl_repo"):
            if p not in sys.path:
                sys.path.insert(0, p)


def _build_program():
    _ensure_path()
    import concourse.bass as bass
    import concourse.mybir as mybir
    import concourse.tile as tile
    from concourse import bacc
    from concourse.masks import make_identity
    from contextlib import ExitStack

    F32 = mybir.dt.float32
    BF16 = mybir.dt.bfloat16
    Alu = mybir.AluOpType
    Act = mybir.ActivationFunctionType

    nc = bacc.Bacc("TRN2", target_bir_lowering=False, debug=False, num_devices=BS)

    hs_d = nc.dram_tensor("hs", [T, D], F32, kind="ExternalInput").ap()
    kgk_d = nc.dram_tensor("kgk", [K, D], F32, kind="ExternalInput").ap()
    kgv_d = nc.dram_tensor("kgv", [K, D], F32, kind="ExternalInput").ap()
    pl_d = nc.dram_tensor("pooled", [1, D], F32, kind="ExternalInput").ap()
    wqt_d = nc.dram_tensor("wqt", [D, D], BF16, kind="ExternalInput").ap()
    wkt_d = nc.dram_tensor("wkt", [D, D], BF16, kind="ExternalInput").ap()
    wvt_d = nc.dram_tensor("wvt", [D, D], BF16, kind="ExternalInput").ap()
    wot_d = nc.dram_tensor("wot", [D, D], BF16, kind="ExternalInput").ap()
    bq_d = nc.dram_tensor("bq", [128, DC], F32, kind="ExternalInput").ap()
    bk_d = nc.dram_tensor("bk", [128, DC], F32, kind="ExternalInput").ap()
    bo_d = nc.dram_tensor("bo", [1, D], F32, kind="ExternalInput").ap()
    beta_d = nc.dram_tensor("beta", [1, H], F32, kind="ExternalInput").ap()
    out_d = nc.dram_tensor("out", [T, D], F32, kind="ExternalOutput").ap()

    with tile.TileContext(nc) as tc, ExitStack() as ctx:
        const = ctx.enter_context(tc.tile_pool(name="const", bufs=1))
        inp = ctx.enter_context(tc.tile_pool(name="inp", bufs=8))
        wpool = ctx.enter_context(tc.tile_pool(name="w", bufs=24))
        big = ctx.enter_context(tc.tile_pool(name="big", bufs=12))
        hstw_p = ctx.enter_context(tc.tile_pool(name="hstw", bufs=12))
        kt_p = ctx.enter_context(tc.tile_pool(name="ktp", bufs=6))
        v_p = ctx.enter_context(tc.tile_pool(name="vp", bufs=4))
        kgt_p = ctx.enter_context(tc.tile_pool(name="kgtp", bufs=6))
        e_p = ctx.enter_context(tc.tile_pool(name="ep", bufs=12))
        r_p = ctx.enter_context(tc.tile_pool(name="rp", bufs=4))
        fin_p = ctx.enter_context(tc.tile_pool(name="finp", bufs=2))
        sm_p = ctx.enter_context(tc.tile_pool(name="smp", bufs=4))
        ps = ctx.enter_context(tc.tile_pool(name="ps", bufs=2, space="PSUM"))

        # ---------------- phase 0: constants + cosine-sim bias ----------------
        ident = const.tile([128, 128], F32, tag="ident")
        make_identity(nc, ident[:])
        ones_bf = const.tile([128, 64], BF16, tag="ones_bf")
        nc.vector.memset(ones_bf[:], 1.0)
        # kg_value loads first: transposes are the critical path into phase 1
        kv_tiles = []
        for c in range(KC):
            kv = inp.tile([128, D], F32, tag="inp", name="kv")
            nc.sync.dma_start(kv[:], kgv_d[c * 128:(c + 1) * 128, :])
            kv_tiles.append(kv)

        # prefetch all projection weights right behind kg_value; kgk and the
        # phase-0 cosine bias only feed the exp bias (needed ~90us in)
        wk_sb = []
        wv_sb = []
        for c in range(DC):
            wk = wpool.tile([128, D], BF16, tag="w")
            nc.sync.dma_start(wk[:], wkt_d[c * 128:(c + 1) * 128, :])
            wk_sb.append(wk)
        for c in range(DC):
            wv = wpool.tile([128, D], BF16, tag="w")
            nc.sync.dma_start(wv[:], wvt_d[c * 128:(c + 1) * 128, :])
            wv_sb.append(wv)
        wq_sb = []
        for c in range(DC):
            wq = wpool.tile([128, D], BF16, tag="w")
            nc.sync.dma_start(wq[:], wqt_d[c * 128:(c + 1) * 128, :])
            wq_sb.append(wq)

        pl = const.tile([1, D], F32, tag="pl")
        nc.sync.dma_start(pl[:], pl_d)
        bt = const.tile([1, H], F32, tag="bt")
        nc.sync.dma_start(bt[:], beta_d)
        bo_row = const.tile([1, D], F32, tag="bo_row")
        nc.sync.dma_start(bo_row[:], bo_d)
        bq_sb = const.tile([128, DC], F32, tag="bq_sb")
        nc.sync.dma_start(bq_sb[:], bq_d)
        bk_sb = const.tile([128, DC], F32, tag="bk_sb")
        nc.sync.dma_start(bk_sb[:], bk_d)

        bo_bc = const.tile([128, D], F32, tag="bo_bc")
        nc.gpsimd.partition_broadcast(bo_bc[:], bo_row[:], channels=128)
        beta_bc = const.tile([128, H], F32, tag="beta_bc")
        nc.gpsimd.partition_broadcast(beta_bc[:], bt[:], channels=128)
        pl_bc = const.tile([128, D], F32, tag="pl_bc")
        nc.gpsimd.partition_broadcast(pl_bc[:], pl[:], channels=128)

        # pooled 1/||.|| as a per-partition vector (computed on the broadcast)
        pl_sq = inp.tile([128, D], F32, tag="inp", name="pl_sq")
        pnorm = sm_p.tile([128, 1], F32, tag="pnorm")
        nc.scalar.activation(pl_sq[:], pl_bc[:], Act.Square, accum_out=pnorm[:])
        nc.scalar.activation(pnorm[:], pnorm[:], Act.Sqrt)
        nc.vector.tensor_scalar_max(pnorm[:], pnorm[:], EPS)
        rp_vec = const.tile([128, 1], F32, tag="rp_vec")
        nc.vector.reciprocal(rp_vec[:], pnorm[:])

        # bias_all[k_part, kc*H + h] = beta[h] * sim[k]
        bias_all = const.tile([128, KC * H], F32, tag="bias_all")
        for c in range(KC):
            kk = inp.tile([128, D], F32, tag="inp")
            nc.sync.dma_start(kk[:], kgk_d[c * 128:(c + 1) * 128, :])
            sq = inp.tile([128, D], F32, tag="inp")
            nrm = sm_p.tile([128, 1], F32, tag="nrm")
            nc.scalar.activation(sq[:], kk[:], Act.Square, accum_out=nrm[:])
            nc.scalar.activation(nrm[:], nrm[:], Act.Sqrt)
            nc.vector.tensor_scalar_max(nrm[:], nrm[:], EPS)
            rn = sm_p.tile([128, 1], F32, tag="rn")
            nc.vector.reciprocal(rn[:], nrm[:])
            sq2 = inp.tile([128, D], F32, tag="inp")
            dot = sm_p.tile([128, 1], F32, tag="dot")
            nc.vector.scalar_tensor_tensor(
                out=sq2[:], in0=kk[:], scalar=1.0, in1=pl_bc[:],
                op0=Alu.mult, op1=Alu.mult, accum_out=dot[:])
            nc.vector.tensor_mul(dot[:], dot[:], rn[:])
            nc.vector.tensor_mul(dot[:], dot[:], rp_vec[:])
            nc.vector.tensor_scalar_mul(
                bias_all[:, c * H:(c + 1) * H], beta_bc[:], dot[:])

        # ---------------- phase 1a: kg_value.T, k.T, v ----------------
        kgt = [kgt_p.tile([128, K], BF16, tag="kgt", name="kgt") for _ in range(DC)]
        for dchunk in range(DC):
            pt = ps.tile([128, K], F32, tag="s", bufs=2, name="ptr")
            for c in range(KC):
                nc.tensor.transpose(
                    pt[:, c * 128:(c + 1) * 128],
                    kv_tiles[c][:, dchunk * 128:(dchunk + 1) * 128], ident[:])
            nc.vector.tensor_copy(kgt[dchunk][:], pt[:])

        kt = [kt_p.tile([128, K], BF16, tag="kt", name="kt") for _ in range(DC)]
        for m in range(DC):
            pk = ps.tile([128, K], F32, tag="mm", bufs=2)
            for c in range(DC):
                nc.tensor.matmul(
                    pk[:], wk_sb[c][:, m * 128:(m + 1) * 128], kgt[c][:],
                    start=(c == 0), stop=(c == DC - 1))
            nc.vector.tensor_scalar_add(kt[m][:], pk[:], bk_sb[:, m:m + 1])

        v_sb = [v_p.tile([128, D], BF16, tag="v", name="vsb")
                for _ in range(KC)]
        for kc in range(KC):
            for n in range(2):
                pv = ps.tile([128, 384], F32, tag="mm", bufs=2)
                for c in range(DC):
                    nc.tensor.matmul(
                        pv[:], kgt[c][:, kc * 128:(kc + 1) * 128],
                        wv_sb[c][:, n * 384:(n + 1) * 384],
                        start=(c == 0), stop=(c == DC - 1))
                nc.vector.tensor_copy(
                    v_sb[kc][:, n * 384:(n + 1) * 384], pv[:])

        # ---------------- phase 1b: hs.T windows + q.T ----------------
        wo_sb = []
        for c in range(DC):
            wo = wpool.tile([128, D], BF16, tag="w")
            nc.sync.dma_start(wo[:], wot_d[c * 128:(c + 1) * 128, :])
            wo_sb.append(wo)

        qt = [big.tile([128, T], BF16, tag="big", name="qt") for _ in range(DC)]
        for tc4 in range(NTW):
            hstw = [hstw_p.tile([128, TW], BF16, tag="hstw", name="hstw") for _ in range(DC)]
            hv_tiles = []
            for tsub in range(TW // 128):
                hv = inp.tile([128, D], F32, tag="inp")
                t0 = tc4 * TW + tsub * 128
                nc.sync.dma_start(hv[:], hs_d[t0:t0 + 128, :])
                hv_tiles.append(hv)
            for c in range(DC):
                pt = ps.tile([128, TW], F32, tag="s", bufs=2, name="ptr")
                for tsub in range(TW // 128):
                    nc.tensor.transpose(
                        pt[:, tsub * 128:(tsub + 1) * 128],
                        hv_tiles[tsub][:, c * 128:(c + 1) * 128], ident[:])
                nc.vector.tensor_copy(hstw[c][:], pt[:])
            for m in range(DC):
                pq = ps.tile([128, TW], F32, tag="mm", bufs=2)
                for c in range(DC):
                    nc.tensor.matmul(
                        pq[:], wq_sb[c][:, m * 128:(m + 1) * 128], hstw[c][:],
                        start=(c == 0), stop=(c == DC - 1))
                nc.vector.tensor_scalar_add(
                    qt[m][:, tc4 * TW:(tc4 + 1) * TW], pq[:], bq_sb[:, m:m + 1])

        # ------- phase 2+3 interleaved: attention + final proj per t-window -------
        ot = [big.tile([128, T], BF16, tag="big", name="ot") for _ in range(NPAIR)]
        for tc4 in range(NTW):
            tw = slice(tc4 * TW, (tc4 + 1) * TW)
            for j in range(NPAIR):
                e_j = []            # [kc] -> (even, odd)
                for kc in range(KC):
                    pse = ps.tile([128, TW], F32, tag="s", bufs=2)
                    nc.tensor.matmul(
                        pse[:], kt[j][0:64, kc * 128:(kc + 1) * 128],
                        qt[j][0:64, tw], start=True, stop=True)
                    pso = ps.tile([128, TW], F32, tag="s", bufs=2)
                    nc.tensor.matmul(
                        pso[:], kt[j][64:128, kc * 128:(kc + 1) * 128],
                        qt[j][64:128, tw], start=True, stop=True)
                    ee = e_p.tile([128, TW], BF16, tag="e")
                    h0 = kc * H + 2 * j
                    nc.scalar.activation(
                        ee[:], pse[:], Act.Exp,
                        bias=bias_all[:, h0:h0 + 1], scale=1.0)
                    eo = e_p.tile([128, TW], BF16, tag="e")
                    nc.scalar.activation(
                        eo[:], pso[:], Act.Exp,
                        bias=bias_all[:, h0 + 1:h0 + 2], scale=1.0)
                    e_j.append((ee, eo))

                # AV + denominator. po holds [evenAV 0:64 ; oddAV 64:128],
                # pd holds [oddDen 0:64 ; evenDen 64:128] (64 copies each).
                # Each v-matmul is paired with a ones-matmul into pd placed
                # on the OPPOSITE array col-group (tile_position), so the
                # two stream the same e tile concurrently — the denominator
                # rides along at no PE-wall cost. All four chains are
                # region-sequential with start=True at kc==0 (safe: start's
                # bank-wide has_written clear only touches bits, not data of
                # chains that already finished).
                po = ps.tile([128, TW], F32, tag="o", bufs=2, name="po")
                pd = ps.tile([128, TW], F32, tag="d", bufs=2, name="pd")
                for kc in range(KC):
                    st = (kc == 0)
                    sp = (kc == KC - 1)
                    nc.tensor.matmul(
                        po[0:64, :],
                        v_sb[kc][:, (2 * j) * HD:(2 * j + 1) * HD],
                        e_j[kc][0][:], start=st, stop=sp)
                    nc.tensor.matmul(
                        pd[64:128, :], ones_bf[:, 0:64],
                        e_j[kc][0][:], start=st, stop=sp,
                        tile_position=(0, 64))
                for kc in range(KC):
                    st = (kc == 0)
                    sp = (kc == KC - 1)
                    nc.tensor.matmul(
                        po[64:128, :],
                        v_sb[kc][:, (2 * j + 1) * HD:(2 * j + 2) * HD],
                        e_j[kc][1][:], start=st, stop=sp)
                    nc.tensor.matmul(
                        pd[0:64, :], ones_bf[:, 0:64],
                        e_j[kc][1][:], start=st, stop=sp,
                        tile_position=(0, 0))

                rall = r_p.tile([128, TW], F32, tag="rall", name="rall")
                nc.vector.reciprocal_approx_fast(rall[:], pd[:])
                nc.vector.tensor_mul(
                    ot[j][0:64, tw], po[0:64, :], rall[64:128, :])
                nc.vector.tensor_mul(
                    ot[j][64:128, tw], po[64:128, :], rall[0:64, :])

            for tsub in range(TW // 128):
                tc16 = tc4 * (TW // 128) + tsub
                fin = fin_p.tile([128, D], F32, tag="fin")
                for n in range(2):
                    pf = ps.tile([128, 384], F32, tag="mm", bufs=2)
                    for c in range(DC):
                        nc.tensor.matmul(
                            pf[:], ot[c][:, tc16 * 128:(tc16 + 1) * 128],
                            wo_sb[c][:, n * 384:(n + 1) * 384],
                            start=(c == 0), stop=(c == DC - 1))
                    nc.vector.tensor_add(
                        fin[:, n * 384:(n + 1) * 384], pf[:],
                        bo_bc[:, n * 384:(n + 1) * 384])
                nc.sync.dma_start(out_d[tc16 * 128:(tc16 + 1) * 128, :], fin[:])

    nc.compile()
    return nc


def _get_program():
    if "nc" not in _CACHE:
        _CACHE["nc"] = _build_program()
    return _CACHE["nc"]


def _host_prep(inputs):
    import ml_dtypes
    bf16 = ml_dtypes.bfloat16

    f32 = lambda x: np.ascontiguousarray(np.asarray(x, dtype=np.float32))
    Wq, Wk, Wv, Wo = (f32(inputs[k]) for k in ("Wq", "Wk", "Wv", "Wo"))
    bq, bk, bv, bo = (f32(inputs[k]) for k in ("bq", "bk", "bv", "bo"))
    beta = f32(inputs["beta"])

    shared = {
        "wqt": np.ascontiguousarray((Wq.T * SCALE).astype(bf16)),
        "wkt": np.ascontiguousarray(Wk.T.astype(bf16)),
        "wvt": np.ascontiguousarray(Wv.T.astype(bf16)),
        "wot": np.ascontiguousarray(Wo.T.astype(bf16)),
        "bq": np.ascontiguousarray((bq * SCALE).reshape(DC, 128).T),
        "bk": np.ascontiguousarray(bk.reshape(DC, 128).T),
        # bv folded through Wo (sum_k softmax == 1), bo absorbed:
        "bo": np.ascontiguousarray((bo + bv @ Wo.T).reshape(1, D)),
        "beta": np.ascontiguousarray(beta.reshape(1, H)),
    }

    hs = f32(inputs["hidden_states"])
    kgk = f32(inputs["kg_key"])
    kgv = f32(inputs["kg_value"])
    pooled = f32(inputs["pooled_hidden_states"])

    in_maps = []
    for b in range(BS):
        m = dict(shared)
        m["hs"] = np.ascontiguousarray(hs[b])
        m["kgk"] = np.ascontiguousarray(kgk[b])
        m["kgv"] = np.ascontiguousarray(kgv[b])
        m["pooled"] = np.ascontiguousarray(pooled[b].reshape(1, D))
        in_maps.append(m)
    return in_maps




def _install_ntff_hook():
    """Register the axon NTFF profile hook so trace=True yields exec_time_ns.

    Only used from our own test harness (TRACE=True); the default kernel()
    path never calls this.
    """
    try:
        from antenv.axon_hooks import get_axon_ntff_profile_hook  # noqa: F401
        return
    except ImportError:
        pass
    import contextlib
    import ctypes
    import types

    so_path = "/opt/axon/libaxon_pjrt.so"
    try:
        lib = ctypes.CDLL(so_path)
    except OSError:
        return
    if not hasattr(lib, "axon_start_nrt_profile"):
        return
    lib.axon_start_nrt_profile.argtypes = [
        ctypes.POINTER(ctypes.c_int64), ctypes.c_size_t]
    lib.axon_start_nrt_profile.restype = ctypes.c_int64
    lib.axon_stop_nrt_profile.argtypes = [ctypes.c_char_p]
    lib.axon_stop_nrt_profile.restype = ctypes.c_int64

    @contextlib.contextmanager
    def _hook(output_dir, device_ids):
        import jax
        jax.devices()
        if device_ids:
            ids = (ctypes.c_int64 * len(device_ids))(*device_ids)
            rc = lib.axon_start_nrt_profile(ids, len(device_ids))
        else:
            rc = lib.axon_start_nrt_profile(None, 0)
        if rc != 0:
            raise RuntimeError(f"axon_start_nrt_profile rc={rc}")
        try:
            yield
        finally:
            n = lib.axon_stop_nrt_profile(str(output_dir).encode())
            print(f"profile: {n} file(s) written to {output_dir}",
                  file=sys.stderr)

    mod = types.ModuleType("antenv.axon_hooks")
    mod.get_axon_ntff_profile_hook = lambda: _hook
    mod.set_axon_ntff_profile_hook = lambda h: None
    sys.modules["antenv.axon_hooks"] = mod


def kernel(**inputs):
    global LAST_EXEC_NS
    _ensure_path()
    from concourse import bass_utils

    if TRACE:
        _install_ntff_hook()
    nc = _get_program()
    in_maps = _host_prep(inputs)
    res = bass_utils.run_bass_kernel_spmd(
        nc, in_maps, core_ids=list(range(BS)), trace=TRACE)
    LAST_EXEC_NS = res.exec_time_ns
    out = np.stack([res.results[b]["out"] for b in range(BS)], axis=0)
    return out.astype(np.float32)



# revision 9
# speedup vs baseline: 1.0700x; 1.0007x over previous
"""Trainium2 Bass kernel for nn_KnowledgeAttention.

Math (per batch example b):
    sim[k]  = cos_sim(pooled[b], kg_key[b,k])                      # [K]
    q       = (hs @ Wq.T + bq) * HD**-0.5     -> heads [T,H,HD]
    k       = kg_value @ Wk.T + bk            -> heads [K,H,HD]
    v       = kg_value @ Wv.T + bv            -> heads [K,H,HD]
    S[h,t,k]= q_h[t]·k_h[k] + beta[h]*sim[k]
    P       = softmax_k(S);  O[t,h] = sum_k P v
    out     = O @ Wo.T + bo

Sharding: pure data-parallel over batch — 8 examples on 8 cores, weights
replicated, no collectives.

Per-core design notes:
  * all matmul contractions run on the partition dim; hs.T / kg_value.T
    via PE transpose.
  * the per-head cosine bias is FACTORED OUT of the exp:
    exp(S + b) = exp(S) * exp(b) with exp(b) folded into v
    (v' = v * exp(b)) and into the denominator matmul stationary
    (ebrep = exp(b) replicated).  This makes the exp bias-free, so one
    ACT op covers the even AND odd head of a pair ([e|o]-batched 1024
    wide, two psum banks), halving ACT op count.
  * scores are computed S.T[k,t] as even/odd row-tiled concurrent
    matmul pairs (stationaries at partition 0:64 / 64:128).
  * AV chains: po = [evenAV ; oddAV]; the denominator matmuls (ebrep
    stationary) go to the opposite array col-group via tile_position so
    they stream the same e tile concurrently with the v matmuls.
  * final projection computed transposed (out.T[dout,t]) so the moving
    operand is ot directly; DRAM output is [D,T], un-transposed on host.
  * matmuls in bf16 with fp32 PSUM accumulation.
"""

import sys

import numpy as np

# ---------------------------------------------------------------- constants
BS = 8
T = 2048
D = 768
H = 12
HD = 64
K = 512
SCALE = HD ** -0.5
EPS = 1e-8
DC = D // 128   # 6 contraction/partition chunks of 128 over D
KC = K // 128   # 4 chunks over K
TW = 512        # t window for moving operand
NTW = T // TW   # 4
NPAIR = H // 2  # 6 head pairs
TB = 1024       # t block for the attention phase
NTB = T // TB   # 2

TRACE = False
LAST_EXEC_NS = None

_CACHE = {}


def _ensure_path():
    try:
        import concourse  # noqa: F401
    except ImportError:
        for p in ("/opt/trn_rl_repo", "/root/.axon_site/_ro/trn_rl_repo"):
            if p not in sys.path:
                sys.path.insert(0, p)


def _build_program():
    _ensure_path()
    import concourse.bass as bass
    import concourse.mybir as mybir
    import concourse.tile as tile
    from concourse import bacc
    from concourse.masks import make_identity
    from contextlib import ExitStack

    F32 = mybir.dt.float32
    BF16 = mybir.dt.bfloat16
    Alu = mybir.AluOpType
    Act = mybir.ActivationFunctionType

    nc = bacc.Bacc("TRN2", target_bir_lowering=False, debug=False, num_devices=BS)

    hs_d = nc.dram_tensor("hs", [T, D], F32, kind="ExternalInput").ap()
    kgk_d = nc.dram_tensor("kgk", [K, D], F32, kind="ExternalInput").ap()
    kgv_d = nc.dram_tensor("kgv", [K, D], F32, kind="ExternalInput").ap()
    pl_d = nc.dram_tensor("pooled", [1, D], F32, kind="ExternalInput").ap()
    wqt_d = nc.dram_tensor("wqt", [D, D], BF16, kind="ExternalInput").ap()
    wkt_d = nc.dram_tensor("wkt", [D, D], BF16, kind="ExternalInput").ap()
    wvt_d = nc.dram_tensor("wvt", [D, D], BF16, kind="ExternalInput").ap()
    wot_d = nc.dram_tensor("wot", [D, D], BF16, kind="ExternalInput").ap()
    bq_d = nc.dram_tensor("bq", [128, DC], F32, kind="ExternalInput").ap()
    bk_d = nc.dram_tensor("bk", [128, DC], F32, kind="ExternalInput").ap()
    bo_d = nc.dram_tensor("bo", [128, DC], F32, kind="ExternalInput").ap()
    beta_d = nc.dram_tensor("beta", [1, H], F32, kind="ExternalInput").ap()
    # output stored transposed [D, T]; host un-transposes
    out_d = nc.dram_tensor("out", [D, T], F32, kind="ExternalOutput").ap()

    with tile.TileContext(nc) as tc, ExitStack() as ctx:
        const = ctx.enter_context(tc.tile_pool(name="const", bufs=1))
        kvkg = ctx.enter_context(tc.tile_pool(name="kvkg", bufs=4))
        inp = ctx.enter_context(tc.tile_pool(name="inp", bufs=8))
        wpool = ctx.enter_context(tc.tile_pool(name="w", bufs=18))
        big = ctx.enter_context(tc.tile_pool(name="big", bufs=12))
        hstw_p = ctx.enter_context(tc.tile_pool(name="hstw", bufs=12))
        kt_p = ctx.enter_context(tc.tile_pool(name="ktp", bufs=6))
        v_p = ctx.enter_context(tc.tile_pool(name="vp", bufs=4))
        kgt_p = ctx.enter_context(tc.tile_pool(name="kgtp", bufs=6))
        e_p = ctx.enter_context(tc.tile_pool(name="ep", bufs=12))
        r_p = ctx.enter_context(tc.tile_pool(name="rp", bufs=4))
        fin_p = ctx.enter_context(tc.tile_pool(name="finp", bufs=3))
        sm_p = ctx.enter_context(tc.tile_pool(name="smp", bufs=4))
        ps = ctx.enter_context(tc.tile_pool(name="ps", bufs=2, space="PSUM"))

        # ---------------- constants + early DMA issue ----------------
        ident = const.tile([128, 128], F32, tag="ident")
        make_identity(nc, ident[:])
        ones_bf = const.tile([128, 64], BF16, tag="ones_bf")
        nc.vector.memset(ones_bf[:], 1.0)

        # kg_value first (transposes are the critical path), split in half
        # column-chunks so two DMA queues work per tile
        kv_tiles = []
        for c in range(KC):
            kv = kvkg.tile([128, D], F32, tag="kvkg", name="kv")
            nc.sync.dma_start(kv[:, 0:384], kgv_d[c * 128:(c + 1) * 128, 0:384])
            nc.sync.dma_start(kv[:, 384:768], kgv_d[c * 128:(c + 1) * 128, 384:768])
            kv_tiles.append(kv)

        wk_sb = []
        wv_sb = []
        wq_sb = []
        for c in range(DC):
            wk = wpool.tile([128, D], BF16, tag="w")
            nc.sync.dma_start(wk[:], wkt_d[c * 128:(c + 1) * 128, :])
            wk_sb.append(wk)
        for c in range(DC):
            wv = wpool.tile([128, D], BF16, tag="w")
            nc.sync.dma_start(wv[:], wvt_d[c * 128:(c + 1) * 128, :])
            wv_sb.append(wv)
        for c in range(DC):
            wq = wpool.tile([128, D], BF16, tag="w")
            nc.sync.dma_start(wq[:], wqt_d[c * 128:(c + 1) * 128, :])
            wq_sb.append(wq)

        pl = const.tile([1, D], F32, tag="pl")
        nc.sync.dma_start(pl[:], pl_d)
        bt = const.tile([1, H], F32, tag="bt")
        nc.sync.dma_start(bt[:], beta_d)
        bq_sb = const.tile([128, DC], F32, tag="bq_sb")
        nc.sync.dma_start(bq_sb[:], bq_d)
        bk_sb = const.tile([128, DC], F32, tag="bk_sb")
        nc.sync.dma_start(bk_sb[:], bk_d)
        bo_sb = const.tile([128, DC], F32, tag="bo_sb")
        nc.sync.dma_start(bo_sb[:], bo_d)

        beta_bc = const.tile([128, H], F32, tag="beta_bc")
        nc.gpsimd.partition_broadcast(beta_bc[:], bt[:], channels=128)
        pl_bc = const.tile([128, D], F32, tag="pl_bc")
        nc.gpsimd.partition_broadcast(pl_bc[:], pl[:], channels=128)

        # ---------------- phase 1a: kg_value.T, k.T, v ----------------
        kgt = [kgt_p.tile([128, K], BF16, tag="kgt", name="kgt") for _ in range(DC)]
        for dchunk in range(DC):
            pt = ps.tile([128, K], F32, tag="od", bufs=4, name="ptr")
            for c in range(KC):
                nc.tensor.transpose(
                    pt[:, c * 128:(c + 1) * 128],
                    kv_tiles[c][:, dchunk * 128:(dchunk + 1) * 128], ident[:])
            nc.vector.tensor_copy(kgt[dchunk][:], pt[:])

        kt = [kt_p.tile([128, K], BF16, tag="kt", name="kt") for _ in range(DC)]
        for m in range(DC):
            pk = ps.tile([128, K], F32, tag="od", bufs=4)
            for c in range(DC):
                nc.tensor.matmul(
                    pk[:], wk_sb[c][:, m * 128:(m + 1) * 128], kgt[c][:],
                    start=(c == 0), stop=(c == DC - 1))
            nc.vector.tensor_scalar_add(kt[m][:], pk[:], bk_sb[:, m:m + 1])

        v_sb = [v_p.tile([128, D], BF16, tag="v", name="vsb")
                for _ in range(KC)]
        for kc in range(KC):
            for n in range(2):
                pv = ps.tile([128, 384], F32, tag="od", bufs=4)
                for c in range(DC):
                    nc.tensor.matmul(
                        pv[:], kgt[c][:, kc * 128:(kc + 1) * 128],
                        wv_sb[c][:, n * 384:(n + 1) * 384],
                        start=(c == 0), stop=(c == DC - 1))
                nc.vector.tensor_copy(
                    v_sb[kc][:, n * 384:(n + 1) * 384], pv[:])

        # ---------------- phase 1b: hs.T windows + q.T ----------------
        wo_sb = []
        for c in range(DC):
            wo = wpool.tile([128, D], BF16, tag="w")
            nc.sync.dma_start(wo[:], wot_d[c * 128:(c + 1) * 128, :])
            wo_sb.append(wo)

        # ebrep/bias tiles declared up-front (filled after tc4==0 below)
        bias_all = const.tile([128, KC * H], F32, tag="bias_all")
        ebv = const.tile([128, KC * H], F32, tag="ebv")
        ebrep = [const.tile([128, H * 64], BF16, tag=f"ebrep{c}", name="ebrep")
                 for c in range(KC)]

        qt = [big.tile([128, T], BF16, tag="big", name="qt") for _ in range(DC)]
        for tc4 in range(NTW):
            hv_tiles = []
            for tsub in range(TW // 128):
                hv = inp.tile([128, D], F32, tag="inp")
                t0 = tc4 * TW + tsub * 128
                nc.sync.dma_start(hv[:, 0:384], hs_d[t0:t0 + 128, 0:384])
                nc.sync.dma_start(hv[:, 384:768], hs_d[t0:t0 + 128, 384:768])
                hv_tiles.append(hv)
            hstw = [hstw_p.tile([128, TW], BF16, tag="hstw", name="hstw")
                    for _ in range(DC)]
            for c in range(DC):
                pt = ps.tile([128, TW], F32, tag="od", bufs=4, name="ptr")
                for tsub in range(TW // 128):
                    nc.tensor.transpose(
                        pt[:, tsub * 128:(tsub + 1) * 128],
                        hv_tiles[tsub][:, c * 128:(c + 1) * 128], ident[:])
                nc.vector.tensor_copy(hstw[c][:], pt[:])
            for m in range(DC):
                pq = ps.tile([128, TW], F32, tag="s", bufs=2)
                for c in range(DC):
                    nc.tensor.matmul(
                        pq[:], wq_sb[c][:, m * 128:(m + 1) * 128], hstw[c][:],
                        start=(c == 0), stop=(c == DC - 1))
                nc.vector.tensor_scalar_add(
                    qt[m][:, tc4 * TW:(tc4 + 1) * TW], pq[:], bq_sb[:, m:m + 1])

            if tc4 == 0:
                # ------- phase 0 (placed here so its DVE/ACT queue slots
                # come after the phase-1a ops they must not block) -------
                pl_sq = inp.tile([128, D], F32, tag="inp", name="pl_sq")
                pnorm = sm_p.tile([128, 1], F32, tag="pnorm")
                nc.scalar.activation(pl_sq[:], pl_bc[:], Act.Square,
                                     accum_out=pnorm[:])
                nc.scalar.activation(pnorm[:], pnorm[:], Act.Sqrt)
                nc.vector.tensor_scalar_max(pnorm[:], pnorm[:], EPS)
                rp_vec = const.tile([128, 1], F32, tag="rp_vec")
                nc.vector.reciprocal(rp_vec[:], pnorm[:])

                for c in range(KC):
                    kk = kvkg.tile([128, D], F32, tag="kvkg", name="kgk")
                    nc.sync.dma_start(kk[:], kgk_d[c * 128:(c + 1) * 128, :])
                    sq = inp.tile([128, D], F32, tag="inp")
                    nrm = sm_p.tile([128, 1], F32, tag="nrm")
                    nc.scalar.activation(sq[:], kk[:], Act.Square,
                                         accum_out=nrm[:])
                    nc.scalar.activation(nrm[:], nrm[:], Act.Sqrt)
                    nc.vector.tensor_scalar_max(nrm[:], nrm[:], EPS)
                    rn = sm_p.tile([128, 1], F32, tag="rn")
                    nc.vector.reciprocal(rn[:], nrm[:])
                    sq2 = inp.tile([128, D], F32, tag="inp")
                    dot = sm_p.tile([128, 1], F32, tag="dot")
                    nc.vector.scalar_tensor_tensor(
                        out=sq2[:], in0=kk[:], scalar=1.0, in1=pl_bc[:],
                        op0=Alu.mult, op1=Alu.mult, accum_out=dot[:])
                    nc.vector.tensor_mul(dot[:], dot[:], rn[:])
                    nc.vector.tensor_mul(dot[:], dot[:], rp_vec[:])
                    nc.vector.tensor_scalar_mul(
                        bias_all[:, c * H:(c + 1) * H], beta_bc[:], dot[:])

                # exp of the bias, then fold into v (v' = v*exp(b)) and
                # build the denominator stationaries (exp(b) replicated)
                nc.scalar.activation(ebv[:], bias_all[:], Act.Exp)
                for c in range(KC):
                    for h in range(H):
                        col = c * H + h
                        nc.vector.tensor_scalar_mul(
                            ebrep[c][:, h * 64:(h + 1) * 64],
                            ones_bf[:, 0:64], ebv[:, col:col + 1])
                        nc.vector.tensor_scalar_mul(
                            v_sb[c][:, h * 64:(h + 1) * 64],
                            v_sb[c][:, h * 64:(h + 1) * 64],
                            ebv[:, col:col + 1])

        # ------- attention + final projection, per t-block of 1024 -------
        ot = [big.tile([128, T], BF16, tag="big", name="ot") for _ in range(NPAIR)]
        for tb in range(NTB):
            for j in range(NPAIR):
                he = 2 * j
                ho = 2 * j + 1
                # scores + batched exp: per (kc, th) one [128,1024] psum
                # tile = [even(512) | odd(512)], one bias-free exp each
                e_jt = [[None] * 2 for _ in range(KC)]
                for kc in range(KC):
                    for th in range(2):
                        tws = slice(tb * TB + th * TW, tb * TB + (th + 1) * TW)
                        pse = ps.tile([128, TB], F32, tag="s", bufs=2)
                        nc.tensor.matmul(
                            pse[:, 0:TW],
                            kt[j][0:64, kc * 128:(kc + 1) * 128],
                            qt[j][0:64, tws], start=True, stop=True)
                        nc.tensor.matmul(
                            pse[:, TW:TB],
                            kt[j][64:128, kc * 128:(kc + 1) * 128],
                            qt[j][64:128, tws], start=True, stop=True)
                        ee = e_p.tile([128, TB], BF16, tag="e")
                        nc.scalar.activation(ee[:], pse[:], Act.Exp)
                        e_jt[kc][th] = ee

                for th in range(2):
                    tws = slice(tb * TB + th * TW, tb * TB + (th + 1) * TW)
                    po = ps.tile([128, TW], F32, tag="od", bufs=4, name="po")
                    pd = ps.tile([128, TW], F32, tag="od", bufs=4, name="pd")
                    for kc in range(KC):
                        st = (kc == 0)
                        sp = (kc == KC - 1)
                        nc.tensor.matmul(
                            po[0:64, :],
                            v_sb[kc][:, he * HD:(he + 1) * HD],
                            e_jt[kc][th][:, 0:TW], start=st, stop=sp)
                        nc.tensor.matmul(
                            pd[64:128, :],
                            ebrep[kc][:, he * HD:(he + 1) * HD],
                            e_jt[kc][th][:, 0:TW], start=st, stop=sp,
                            tile_position=(0, 64))
                    for kc in range(KC):
                        st = (kc == 0)
                        sp = (kc == KC - 1)
                        nc.tensor.matmul(
                            po[64:128, :],
                            v_sb[kc][:, ho * HD:(ho + 1) * HD],
                            e_jt[kc][th][:, TW:TB], start=st, stop=sp)
                        nc.tensor.matmul(
                            pd[0:64, :],
                            ebrep[kc][:, ho * HD:(ho + 1) * HD],
                            e_jt[kc][th][:, TW:TB], start=st, stop=sp,
                            tile_position=(0, 0))

                    rall = r_p.tile([128, TW], F32, tag="rall", name="rall")
                    nc.vector.reciprocal_approx_fast(rall[:], pd[:])
                    nc.vector.tensor_mul(
                        ot[j][0:64, tws], po[0:64, :], rall[64:128, :])
                    nc.vector.tensor_mul(
                        ot[j][64:128, tws], po[64:128, :], rall[0:64, :])

            # final projection for this t-block, transposed: out.T[dout, t]
            for m in range(DC):
                for th in range(2):
                    tws = slice(tb * TB + th * TW, tb * TB + (th + 1) * TW)
                    pf = ps.tile([128, TW], F32, tag="od", bufs=4)
                    for c in range(DC):
                        nc.tensor.matmul(
                            pf[:], wo_sb[c][:, m * 128:(m + 1) * 128],
                            ot[c][:, tws],
                            start=(c == 0), stop=(c == DC - 1))
                    fin = fin_p.tile([128, TW], F32, tag="fin")
                    nc.vector.tensor_scalar_add(
                        fin[:], pf[:], bo_sb[:, m:m + 1])
                    nc.sync.dma_start(
                        out_d[m * 128:(m + 1) * 128, tws], fin[:])

    nc.compile()
    return nc


def _get_program():
    if "nc" not in _CACHE:
        _CACHE["nc"] = _build_program()
    return _CACHE["nc"]


def _host_prep(inputs):
    import ml_dtypes
    bf16 = ml_dtypes.bfloat16

    f32 = lambda x: np.ascontiguousarray(np.asarray(x, dtype=np.float32))
    Wq, Wk, Wv, Wo = (f32(inputs[k]) for k in ("Wq", "Wk", "Wv", "Wo"))
    bq, bk, bv, bo = (f32(inputs[k]) for k in ("bq", "bk", "bv", "bo"))
    beta = f32(inputs["beta"])

    shared = {
        "wqt": np.ascontiguousarray((Wq.T * SCALE).astype(bf16)),
        "wkt": np.ascontiguousarray(Wk.T.astype(bf16)),
        "wvt": np.ascontiguousarray(Wv.T.astype(bf16)),
        "wot": np.ascontiguousarray(Wo.T.astype(bf16)),
        "bq": np.ascontiguousarray((bq * SCALE).reshape(DC, 128).T),
        "bk": np.ascontiguousarray(bk.reshape(DC, 128).T),
        # bv folded through Wo (sum_k softmax == 1), bo absorbed:
        "bo": np.ascontiguousarray((bo + bv @ Wo.T).reshape(DC, 128).T),
        "beta": np.ascontiguousarray(beta.reshape(1, H)),
    }

    hs = f32(inputs["hidden_states"])
    kgk = f32(inputs["kg_key"])
    kgv = f32(inputs["kg_value"])
    pooled = f32(inputs["pooled_hidden_states"])

    in_maps = []
    for b in range(BS):
        m = dict(shared)
        m["hs"] = np.ascontiguousarray(hs[b])
        m["kgk"] = np.ascontiguousarray(kgk[b])
        m["kgv"] = np.ascontiguousarray(kgv[b])
        m["pooled"] = np.ascontiguousarray(pooled[b].reshape(1, D))
        in_maps.append(m)
    return in_maps




def _install_ntff_hook():
    """Register the axon NTFF profile hook so trace=True yields exec_time_ns.

    Only used from our own test harness (TRACE=True); the default kernel()
    path never calls this.
    """
    try:
        from antenv.axon_hooks import get_axon_ntff_profile_hook  # noqa: F401
        return
    except ImportError:
        pass
    import contextlib
    import ctypes
    import types

    so_path = "/opt/axon/libaxon_pjrt.so"
    try:
        lib = ctypes.CDLL(so_path)
    except OSError:
        return
    if not hasattr(lib, "axon_start_nrt_profile"):
        return
    lib.axon_start_nrt_profile.argtypes = [
        ctypes.POINTER(ctypes.c_int64), ctypes.c_size_t]
    lib.axon_start_nrt_profile.restype = ctypes.c_int64
    lib.axon_stop_nrt_profile.argtypes = [ctypes.c_char_p]
    lib.axon_stop_nrt_profile.restype = ctypes.c_int64

    @contextlib.contextmanager
    def _hook(output_dir, device_ids):
        import jax
        jax.devices()
        if device_ids:
            ids = (ctypes.c_int64 * len(device_ids))(*device_ids)
            rc = lib.axon_start_nrt_profile(ids, len(device_ids))
        else:
            rc = lib.axon_start_nrt_profile(None, 0)
        if rc != 0:
            raise RuntimeError(f"axon_start_nrt_profile rc={rc}")
        try:
            yield
        finally:
            n = lib.axon_stop_nrt_profile(str(output_dir).encode())
            print(f"profile: {n} file(s) written to {output_dir}",
                  file=sys.stderr)

    mod = types.ModuleType("antenv.axon_hooks")
    mod.get_axon_ntff_profile_hook = lambda: _hook
    mod.set_axon_ntff_profile_hook = lambda h: None
    sys.modules["antenv.axon_hooks"] = mod


def kernel(**inputs):
    global LAST_EXEC_NS
    _ensure_path()
    from concourse import bass_utils

    if TRACE:
        _install_ntff_hook()
    nc = _get_program()
    in_maps = _host_prep(inputs)
    res = bass_utils.run_bass_kernel_spmd(
        nc, in_maps, core_ids=list(range(BS)), trace=TRACE)
    LAST_EXEC_NS = res.exec_time_ns
    # device output is out.T [D, T]; un-transpose per example
    out = np.stack([res.results[b]["out"].T for b in range(BS)], axis=0)
    return np.ascontiguousarray(out).astype(np.float32)


# revision 10
# speedup vs baseline: 1.3344x; 1.2470x over previous
"""Trainium2 Bass kernel for nn_KnowledgeAttention.

Math (per batch example b):
    sim[k]  = cos_sim(pooled[b], kg_key[b,k])                      # [K]
    q       = (hs @ Wq.T + bq) * HD**-0.5     -> heads [T,H,HD]
    k       = kg_value @ Wk.T + bk            -> heads [K,H,HD]
    v       = kg_value @ Wv.T + bv            -> heads [K,H,HD]
    S[h,t,k]= q_h[t]·k_h[k] + beta[h]*sim[k]
    P       = softmax_k(S);  O[t,h] = sum_k P v
    out     = O @ Wo.T + bo

Sharding: pure data-parallel over batch — 8 examples on 8 cores, weights
replicated, no collectives.

Per-core design notes:
  * all matmul contractions run on the partition dim; hs.T / kg_value.T
    via PE transpose.
  * the per-head cosine bias is FACTORED OUT of the exp:
    exp(S + b) = exp(S) * exp(b) with exp(b) folded into v
    (v' = v * exp(b)) and into the denominator matmul stationary
    (ebrep = exp(b) replicated).  This makes the exp bias-free, so one
    ACT op covers the even AND odd head of a pair ([e|o]-batched 1024
    wide, two psum banks), halving ACT op count.
  * scores are computed S.T[k,t] as even/odd row-tiled concurrent
    matmul pairs (stationaries at partition 0:64 / 64:128).
  * AV chains: po = [evenAV ; oddAV]; the denominator matmuls (ebrep
    stationary) go to the opposite array col-group via tile_position so
    they stream the same e tile concurrently with the v matmuls.
  * final projection computed transposed (out.T[dout,t]) so the moving
    operand is ot directly; DRAM output is [D,T], un-transposed on host.
  * matmuls in bf16 with fp32 PSUM accumulation.
"""

import sys

import numpy as np

# ---------------------------------------------------------------- constants
BS = 8
T = 2048
D = 768
H = 12
HD = 64
K = 512
SCALE = HD ** -0.5
EPS = 1e-8
DC = D // 128   # 6 contraction/partition chunks of 128 over D
KC = K // 128   # 4 chunks over K
TW = 512        # t window for moving operand
NTW = T // TW   # 4
NPAIR = H // 2  # 6 head pairs
TB = 1024       # t block for the attention phase
NTB = T // TB   # 2

TRACE = False
LAST_EXEC_NS = None

_CACHE = {}


def _ensure_path():
    try:
        import concourse  # noqa: F401
    except ImportError:
        for p in ("/opt/trn_rl_repo", "/root/.axon_site/_ro/trn_rl_repo"):
            if p not in sys.path:
                sys.path.insert(0, p)


def _build_program():
    _ensure_path()
    import concourse.bass as bass
    import concourse.mybir as mybir
    import concourse.tile as tile
    from concourse import bacc
    from concourse.masks import make_identity
    from contextlib import ExitStack

    F32 = mybir.dt.float32
    BF16 = mybir.dt.bfloat16
    Alu = mybir.AluOpType
    Act = mybir.ActivationFunctionType

    nc = bacc.Bacc("TRN2", target_bir_lowering=False, debug=False, num_devices=BS)

    hs_d = nc.dram_tensor("hs", [T, D], F32, kind="ExternalInput").ap()
    kgk_d = nc.dram_tensor("kgk", [K, D], F32, kind="ExternalInput").ap()
    kgv_d = nc.dram_tensor("kgv", [K, D], F32, kind="ExternalInput").ap()
    pl_d = nc.dram_tensor("pooled", [1, D], F32, kind="ExternalInput").ap()
    wqt_d = nc.dram_tensor("wqt", [D, D], BF16, kind="ExternalInput").ap()
    wkt_d = nc.dram_tensor("wkt", [D, D], BF16, kind="ExternalInput").ap()
    wvt_d = nc.dram_tensor("wvt", [D, D], BF16, kind="ExternalInput").ap()
    wot_d = nc.dram_tensor("wot", [D, D], BF16, kind="ExternalInput").ap()
    bq_d = nc.dram_tensor("bq", [128, DC], F32, kind="ExternalInput").ap()
    bk_d = nc.dram_tensor("bk", [128, DC], F32, kind="ExternalInput").ap()
    bo_d = nc.dram_tensor("bo", [128, DC], F32, kind="ExternalInput").ap()
    beta_d = nc.dram_tensor("beta", [1, H], F32, kind="ExternalInput").ap()
    # output stored transposed [D, T]; host un-transposes
    out_d = nc.dram_tensor("out", [D, T], F32, kind="ExternalOutput").ap()

    with tile.TileContext(nc) as tc, ExitStack() as ctx:
        const = ctx.enter_context(tc.tile_pool(name="const", bufs=1))
        kvkg = ctx.enter_context(tc.tile_pool(name="kvkg", bufs=4))
        inp = ctx.enter_context(tc.tile_pool(name="inp", bufs=8))
        wpool = ctx.enter_context(tc.tile_pool(name="w", bufs=18))
        big = ctx.enter_context(tc.tile_pool(name="big", bufs=12))
        hstw_p = ctx.enter_context(tc.tile_pool(name="hstw", bufs=12))
        kt_p = ctx.enter_context(tc.tile_pool(name="ktp", bufs=6))
        v_p = ctx.enter_context(tc.tile_pool(name="vp", bufs=4))
        kgt_p = ctx.enter_context(tc.tile_pool(name="kgtp", bufs=6))
        e_p = ctx.enter_context(tc.tile_pool(name="ep", bufs=12))
        r_p = ctx.enter_context(tc.tile_pool(name="rp", bufs=4))
        fin_p = ctx.enter_context(tc.tile_pool(name="finp", bufs=3))
        sm_p = ctx.enter_context(tc.tile_pool(name="smp", bufs=4))
        ps = ctx.enter_context(tc.tile_pool(name="ps", bufs=2, space="PSUM"))

        # ---------------- constants + early DMA issue ----------------
        ident = const.tile([128, 128], F32, tag="ident")
        make_identity(nc, ident[:])
        ones_bf = const.tile([128, 64], BF16, tag="ones_bf")
        nc.vector.memset(ones_bf[:], 1.0)

        # kg_value first (transposes are the critical path), split in half
        # column-chunks so two DMA queues work per tile
        kv_tiles = []
        for c in range(KC):
            kv = kvkg.tile([128, D], F32, tag="kvkg", name="kv")
            nc.sync.dma_start(kv[:, 0:384], kgv_d[c * 128:(c + 1) * 128, 0:384])
            nc.sync.dma_start(kv[:, 384:768], kgv_d[c * 128:(c + 1) * 128, 384:768])
            kv_tiles.append(kv)

        wk_sb = []
        wv_sb = []
        wq_sb = []
        for c in range(DC):
            wk = wpool.tile([128, D], BF16, tag="w")
            nc.sync.dma_start(wk[:], wkt_d[c * 128:(c + 1) * 128, :])
            wk_sb.append(wk)
        for c in range(DC):
            wv = wpool.tile([128, D], BF16, tag="w")
            nc.sync.dma_start(wv[:], wvt_d[c * 128:(c + 1) * 128, :])
            wv_sb.append(wv)
        for c in range(DC):
            wq = wpool.tile([128, D], BF16, tag="w")
            nc.sync.dma_start(wq[:], wqt_d[c * 128:(c + 1) * 128, :])
            wq_sb.append(wq)

        pl = const.tile([1, D], F32, tag="pl")
        nc.sync.dma_start(pl[:], pl_d)
        bt = const.tile([1, H], F32, tag="bt")
        nc.sync.dma_start(bt[:], beta_d)
        bq_sb = const.tile([128, DC], F32, tag="bq_sb")
        nc.sync.dma_start(bq_sb[:], bq_d)
        bk_sb = const.tile([128, DC], F32, tag="bk_sb")
        nc.sync.dma_start(bk_sb[:], bk_d)
        bo_sb = const.tile([128, DC], F32, tag="bo_sb")
        nc.sync.dma_start(bo_sb[:], bo_d)

        beta_bc = const.tile([128, H], F32, tag="beta_bc")
        nc.gpsimd.partition_broadcast(beta_bc[:], bt[:], channels=128)
        pl_bc = const.tile([128, D], F32, tag="pl_bc")
        nc.gpsimd.partition_broadcast(pl_bc[:], pl[:], channels=128)

        # ---------------- phase 1a: kg_value.T, k.T, v ----------------
        kgt = [kgt_p.tile([128, K], BF16, tag="kgt", name="kgt") for _ in range(DC)]
        for dchunk in range(DC):
            pt = ps.tile([128, K], F32, tag="od", bufs=4, name="ptr")
            for c in range(KC):
                nc.tensor.transpose(
                    pt[:, c * 128:(c + 1) * 128],
                    kv_tiles[c][:, dchunk * 128:(dchunk + 1) * 128], ident[:])
            nc.vector.tensor_copy(kgt[dchunk][:], pt[:])

        kt = [kt_p.tile([128, K], BF16, tag="kt", name="kt") for _ in range(DC)]
        for m in range(DC):
            pk = ps.tile([128, K], F32, tag="od", bufs=4)
            for c in range(DC):
                nc.tensor.matmul(
                    pk[:], wk_sb[c][:, m * 128:(m + 1) * 128], kgt[c][:],
                    start=(c == 0), stop=(c == DC - 1))
            nc.vector.tensor_scalar_add(kt[m][:], pk[:], bk_sb[:, m:m + 1])

        v_sb = [v_p.tile([128, D], BF16, tag="v", name="vsb")
                for _ in range(KC)]
        for kc in range(KC):
            for n in range(2):
                pv = ps.tile([128, 384], F32, tag="od", bufs=4)
                for c in range(DC):
                    nc.tensor.matmul(
                        pv[:], kgt[c][:, kc * 128:(kc + 1) * 128],
                        wv_sb[c][:, n * 384:(n + 1) * 384],
                        start=(c == 0), stop=(c == DC - 1))
                nc.vector.tensor_copy(
                    v_sb[kc][:, n * 384:(n + 1) * 384], pv[:])

        # ---------------- phase 1b: hs.T windows + q.T ----------------
        wo_sb = []
        for c in range(DC):
            wo = wpool.tile([128, D], BF16, tag="w")
            nc.sync.dma_start(wo[:], wot_d[c * 128:(c + 1) * 128, :])
            wo_sb.append(wo)

        # ebrep/bias tiles declared up-front (filled after tc4==0 below)
        bias_all = const.tile([128, KC * H], F32, tag="bias_all")
        ebv = const.tile([128, KC * H], F32, tag="ebv")
        ebrep = [const.tile([128, H * 64], BF16, tag=f"ebrep{c}", name="ebrep")
                 for c in range(KC)]

        qt = [big.tile([128, T], BF16, tag="big", name="qt") for _ in range(DC)]
        for tc4 in range(NTW):
            hv_tiles = []
            for tsub in range(TW // 128):
                hv = inp.tile([128, D], F32, tag="inp")
                t0 = tc4 * TW + tsub * 128
                nc.sync.dma_start(hv[:, 0:384], hs_d[t0:t0 + 128, 0:384])
                nc.sync.dma_start(hv[:, 384:768], hs_d[t0:t0 + 128, 384:768])
                hv_tiles.append(hv)
            hstw = [hstw_p.tile([128, TW], BF16, tag="hstw", name="hstw")
                    for _ in range(DC)]
            for c in range(DC):
                pt = ps.tile([128, TW], F32, tag="od", bufs=4, name="ptr")
                for tsub in range(TW // 128):
                    nc.tensor.transpose(
                        pt[:, tsub * 128:(tsub + 1) * 128],
                        hv_tiles[tsub][:, c * 128:(c + 1) * 128], ident[:])
                nc.vector.tensor_copy(hstw[c][:], pt[:])
            for m in range(DC):
                pq = ps.tile([128, TW], F32, tag="s", bufs=2)
                for c in range(DC):
                    nc.tensor.matmul(
                        pq[:], wq_sb[c][:, m * 128:(m + 1) * 128], hstw[c][:],
                        start=(c == 0), stop=(c == DC - 1))
                nc.vector.tensor_scalar_add(
                    qt[m][:, tc4 * TW:(tc4 + 1) * TW], pq[:], bq_sb[:, m:m + 1])

            if tc4 == 0:
                # ------- phase 0 (placed here so its DVE/ACT queue slots
                # come after the phase-1a ops they must not block) -------
                pl_sq = inp.tile([128, D], F32, tag="inp", name="pl_sq")
                pnorm = sm_p.tile([128, 1], F32, tag="pnorm")
                nc.scalar.activation(pl_sq[:], pl_bc[:], Act.Square,
                                     accum_out=pnorm[:])
                nc.scalar.activation(pnorm[:], pnorm[:], Act.Sqrt)
                nc.vector.tensor_scalar_max(pnorm[:], pnorm[:], EPS)
                rp_vec = const.tile([128, 1], F32, tag="rp_vec")
                nc.vector.reciprocal(rp_vec[:], pnorm[:])

                for c in range(KC):
                    kk = kvkg.tile([128, D], F32, tag="kvkg", name="kgk")
                    nc.sync.dma_start(kk[:], kgk_d[c * 128:(c + 1) * 128, :])
                    sq = inp.tile([128, D], F32, tag="inp")
                    nrm = sm_p.tile([128, 1], F32, tag="nrm")
                    nc.scalar.activation(sq[:], kk[:], Act.Square,
                                         accum_out=nrm[:])
                    nc.scalar.activation(nrm[:], nrm[:], Act.Sqrt)
                    nc.vector.tensor_scalar_max(nrm[:], nrm[:], EPS)
                    rn = sm_p.tile([128, 1], F32, tag="rn")
                    nc.vector.reciprocal(rn[:], nrm[:])
                    sq2 = inp.tile([128, D], F32, tag="inp")
                    dot = sm_p.tile([128, 1], F32, tag="dot")
                    nc.vector.scalar_tensor_tensor(
                        out=sq2[:], in0=kk[:], scalar=1.0, in1=pl_bc[:],
                        op0=Alu.mult, op1=Alu.mult, accum_out=dot[:])
                    nc.vector.tensor_mul(dot[:], dot[:], rn[:])
                    nc.vector.tensor_mul(dot[:], dot[:], rp_vec[:])
                    nc.vector.tensor_scalar_mul(
                        bias_all[:, c * H:(c + 1) * H], beta_bc[:], dot[:])

                # exp of the bias, then fold into v (v' = v*exp(b)) and
                # build the denominator stationaries (exp(b) replicated).
                # These run on the SCALAR engine (per-partition scale arg):
                # DVE has no slack in phase 1b, ACT does.
                nc.scalar.activation(ebv[:], bias_all[:], Act.Exp)
                for c in range(KC):
                    for h in range(H):
                        col = c * H + h
                        nc.scalar.activation(
                            ebrep[c][:, h * 64:(h + 1) * 64],
                            ones_bf[:, 0:64], Act.Copy,
                            scale=ebv[:, col:col + 1])
                        nc.scalar.activation(
                            v_sb[c][:, h * 64:(h + 1) * 64],
                            v_sb[c][:, h * 64:(h + 1) * 64], Act.Copy,
                            scale=ebv[:, col:col + 1])

        # ------- attention + final projection, per t-block of 1024 -------
        ot = [big.tile([128, T], BF16, tag="big", name="ot") for _ in range(NPAIR)]
        for tb in range(NTB):
            for j in range(NPAIR):
                he = 2 * j
                ho = 2 * j + 1
                # scores + batched exp: per (kc, th) one [128,1024] psum
                # tile = [even(512) | odd(512)], one bias-free exp each
                e_jt = [[None] * 2 for _ in range(KC)]
                for kc in range(KC):
                    for th in range(2):
                        tws = slice(tb * TB + th * TW, tb * TB + (th + 1) * TW)
                        pse = ps.tile([128, TB], F32, tag="s", bufs=2)
                        nc.tensor.matmul(
                            pse[:, 0:TW],
                            kt[j][0:64, kc * 128:(kc + 1) * 128],
                            qt[j][0:64, tws], start=True, stop=True)
                        nc.tensor.matmul(
                            pse[:, TW:TB],
                            kt[j][64:128, kc * 128:(kc + 1) * 128],
                            qt[j][64:128, tws], start=True, stop=True)
                        ee = e_p.tile([128, TB], BF16, tag="e")
                        nc.scalar.activation(ee[:], pse[:], Act.Exp)
                        e_jt[kc][th] = ee

                for th in range(2):
                    tws = slice(tb * TB + th * TW, tb * TB + (th + 1) * TW)
                    po = ps.tile([128, TW], F32, tag="od", bufs=4, name="po")
                    pd = ps.tile([128, TW], F32, tag="od", bufs=4, name="pd")
                    for kc in range(KC):
                        st = (kc == 0)
                        sp = (kc == KC - 1)
                        nc.tensor.matmul(
                            po[0:64, :],
                            v_sb[kc][:, he * HD:(he + 1) * HD],
                            e_jt[kc][th][:, 0:TW], start=st, stop=sp)
                        nc.tensor.matmul(
                            pd[64:128, :],
                            ebrep[kc][:, he * HD:(he + 1) * HD],
                            e_jt[kc][th][:, 0:TW], start=st, stop=sp,
                            tile_position=(0, 64))
                    for kc in range(KC):
                        st = (kc == 0)
                        sp = (kc == KC - 1)
                        nc.tensor.matmul(
                            po[64:128, :],
                            v_sb[kc][:, ho * HD:(ho + 1) * HD],
                            e_jt[kc][th][:, TW:TB], start=st, stop=sp)
                        nc.tensor.matmul(
                            pd[0:64, :],
                            ebrep[kc][:, ho * HD:(ho + 1) * HD],
                            e_jt[kc][th][:, TW:TB], start=st, stop=sp,
                            tile_position=(0, 0))

                    rall = r_p.tile([128, TW], F32, tag="rall", name="rall")
                    nc.vector.reciprocal_approx_fast(rall[:], pd[:])
                    nc.vector.tensor_mul(
                        ot[j][0:64, tws], po[0:64, :], rall[64:128, :])
                    nc.vector.tensor_mul(
                        ot[j][64:128, tws], po[64:128, :], rall[0:64, :])

            # final projection for this t-block, transposed: out.T[dout, t]
            for m in range(DC):
                for th in range(2):
                    tws = slice(tb * TB + th * TW, tb * TB + (th + 1) * TW)
                    pf = ps.tile([128, TW], F32, tag="od", bufs=4)
                    for c in range(DC):
                        nc.tensor.matmul(
                            pf[:], wo_sb[c][:, m * 128:(m + 1) * 128],
                            ot[c][:, tws],
                            start=(c == 0), stop=(c == DC - 1))
                    fin = fin_p.tile([128, TW], F32, tag="fin")
                    nc.vector.tensor_scalar_add(
                        fin[:], pf[:], bo_sb[:, m:m + 1])
                    nc.sync.dma_start(
                        out_d[m * 128:(m + 1) * 128, tws], fin[:])

    nc.compile()
    return nc


def _get_program():
    if "nc" not in _CACHE:
        _CACHE["nc"] = _build_program()
    return _CACHE["nc"]


def _host_prep(inputs):
    import ml_dtypes
    bf16 = ml_dtypes.bfloat16

    f32 = lambda x: np.ascontiguousarray(np.asarray(x, dtype=np.float32))
    Wq, Wk, Wv, Wo = (f32(inputs[k]) for k in ("Wq", "Wk", "Wv", "Wo"))
    bq, bk, bv, bo = (f32(inputs[k]) for k in ("bq", "bk", "bv", "bo"))
    beta = f32(inputs["beta"])

    shared = {
        "wqt": np.ascontiguousarray((Wq.T * SCALE).astype(bf16)),
        "wkt": np.ascontiguousarray(Wk.T.astype(bf16)),
        "wvt": np.ascontiguousarray(Wv.T.astype(bf16)),
        "wot": np.ascontiguousarray(Wo.T.astype(bf16)),
        "bq": np.ascontiguousarray((bq * SCALE).reshape(DC, 128).T),
        "bk": np.ascontiguousarray(bk.reshape(DC, 128).T),
        # bv folded through Wo (sum_k softmax == 1), bo absorbed:
        "bo": np.ascontiguousarray((bo + bv @ Wo.T).reshape(DC, 128).T),
        "beta": np.ascontiguousarray(beta.reshape(1, H)),
    }

    hs = f32(inputs["hidden_states"])
    kgk = f32(inputs["kg_key"])
    kgv = f32(inputs["kg_value"])
    pooled = f32(inputs["pooled_hidden_states"])

    in_maps = []
    for b in range(BS):
        m = dict(shared)
        m["hs"] = np.ascontiguousarray(hs[b])
        m["kgk"] = np.ascontiguousarray(kgk[b])
        m["kgv"] = np.ascontiguousarray(kgv[b])
        m["pooled"] = np.ascontiguousarray(pooled[b].reshape(1, D))
        in_maps.append(m)
    return in_maps




def _install_ntff_hook():
    """Register the axon NTFF profile hook so trace=True yields exec_time_ns.

    Only used from our own test harness (TRACE=True); the default kernel()
    path never calls this.
    """
    try:
        from antenv.axon_hooks import get_axon_ntff_profile_hook  # noqa: F401
        return
    except ImportError:
        pass
    import contextlib
    import ctypes
    import types

    so_path = "/opt/axon/libaxon_pjrt.so"
    try:
        lib = ctypes.CDLL(so_path)
    except OSError:
        return
    if not hasattr(lib, "axon_start_nrt_profile"):
        return
    lib.axon_start_nrt_profile.argtypes = [
        ctypes.POINTER(ctypes.c_int64), ctypes.c_size_t]
    lib.axon_start_nrt_profile.restype = ctypes.c_int64
    lib.axon_stop_nrt_profile.argtypes = [ctypes.c_char_p]
    lib.axon_stop_nrt_profile.restype = ctypes.c_int64

    @contextlib.contextmanager
    def _hook(output_dir, device_ids):
        import jax
        jax.devices()
        if device_ids:
            ids = (ctypes.c_int64 * len(device_ids))(*device_ids)
            rc = lib.axon_start_nrt_profile(ids, len(device_ids))
        else:
            rc = lib.axon_start_nrt_profile(None, 0)
        if rc != 0:
            raise RuntimeError(f"axon_start_nrt_profile rc={rc}")
        try:
            yield
        finally:
            n = lib.axon_stop_nrt_profile(str(output_dir).encode())
            print(f"profile: {n} file(s) written to {output_dir}",
                  file=sys.stderr)

    mod = types.ModuleType("antenv.axon_hooks")
    mod.get_axon_ntff_profile_hook = lambda: _hook
    mod.set_axon_ntff_profile_hook = lambda h: None
    sys.modules["antenv.axon_hooks"] = mod


def kernel(**inputs):
    global LAST_EXEC_NS
    _ensure_path()
    from concourse import bass_utils

    if TRACE:
        _install_ntff_hook()
    nc = _get_program()
    in_maps = _host_prep(inputs)
    res = bass_utils.run_bass_kernel_spmd(
        nc, in_maps, core_ids=list(range(BS)), trace=TRACE)
    LAST_EXEC_NS = res.exec_time_ns
    # device output is out.T [D, T]; un-transpose per example
    out = np.stack([res.results[b]["out"].T for b in range(BS)], axis=0)
    return np.ascontiguousarray(out).astype(np.float32)


# revision 11
# speedup vs baseline: 1.3606x; 1.0197x over previous
"""Trainium2 Bass kernel for nn_KnowledgeAttention.

Math (per batch example b):
    sim[k]  = cos_sim(pooled[b], kg_key[b,k])                      # [K]
    q       = (hs @ Wq.T + bq) * HD**-0.5     -> heads [T,H,HD]
    k       = kg_value @ Wk.T + bk            -> heads [K,H,HD]
    v       = kg_value @ Wv.T + bv            -> heads [K,H,HD]
    S[h,t,k]= q_h[t]·k_h[k] + beta[h]*sim[k]
    P       = softmax_k(S);  O[t,h] = sum_k P v
    out     = O @ Wo.T + bo

Sharding: pure data-parallel over batch — 8 examples on 8 cores, weights
replicated, no collectives.

Per-core design notes:
  * hs / kg_value / kg_key are shipped bf16 from the host; hs.T and
    kg_value.T land in SBUF directly via DMA-xbar transpose loads
    (dma_start_transpose) — no PE transposes, no psum->sbuf copies.
  * the per-head cosine bias is FACTORED OUT of the exp:
    exp(S + b) = exp(S) * exp(b); exp(b) is folded into v (applied
    during the v-projection psum->sbuf copy, on the scalar engine with
    a per-partition scale) and into the denominator matmul stationary
    (ebrep = exp(b) replicated 64 wide).  The exp is then bias-free, so
    one ACT op covers the even AND odd head of a pair ([e|o]-batched,
    1024 wide across two psum banks).
  * scores are computed S.T[k,t] as even/odd row-tiled concurrent
    matmul pairs (stationaries at partition 0:64 / 64:128).
  * AV chains: po = [evenAV ; oddAV]; the denominator matmuls (ebrep
    stationary) go to the opposite array col-group via tile_position so
    they stream the same e tile concurrently with the v matmuls.
  * final projection computed transposed (out.T[dout,t]) so the moving
    operand is ot directly; DRAM output is [D,T], un-transposed on host.
  * matmuls in bf16 with fp32 PSUM accumulation.
"""

import sys

import numpy as np

# ---------------------------------------------------------------- constants
BS = 8
T = 2048
D = 768
H = 12
HD = 64
K = 512
SCALE = HD ** -0.5
EPS = 1e-8
DC = D // 128   # 6 contraction/partition chunks of 128 over D
KC = K // 128   # 4 chunks over K
TW = 512        # t window (psum free-dim limit)
NTW = T // TW   # 4
NPAIR = H // 2  # 6 head pairs

TRACE = False
LAST_EXEC_NS = None

_CACHE = {}


def _ensure_path():
    try:
        import concourse  # noqa: F401
    except ImportError:
        for p in ("/opt/trn_rl_repo", "/root/.axon_site/_ro/trn_rl_repo"):
            if p not in sys.path:
                sys.path.insert(0, p)


def _build_program():
    _ensure_path()
    import concourse.bass as bass
    import concourse.mybir as mybir
    import concourse.tile as tile
    from concourse import bacc
    from contextlib import ExitStack

    F32 = mybir.dt.float32
    BF16 = mybir.dt.bfloat16
    Alu = mybir.AluOpType
    Act = mybir.ActivationFunctionType

    nc = bacc.Bacc("TRN2", target_bir_lowering=False, debug=False, num_devices=BS)

    hs_d = nc.dram_tensor("hs", [T, D], BF16, kind="ExternalInput").ap()
    kgk_d = nc.dram_tensor("kgk", [K, D], BF16, kind="ExternalInput").ap()
    kgv_d = nc.dram_tensor("kgv", [K, D], BF16, kind="ExternalInput").ap()
    pl_d = nc.dram_tensor("pooled", [1, D], F32, kind="ExternalInput").ap()
    wqt_d = nc.dram_tensor("wqt", [D, D], BF16, kind="ExternalInput").ap()
    wkt_d = nc.dram_tensor("wkt", [D, D], BF16, kind="ExternalInput").ap()
    wvt_d = nc.dram_tensor("wvt", [D, D], BF16, kind="ExternalInput").ap()
    wot_d = nc.dram_tensor("wot", [D, D], BF16, kind="ExternalInput").ap()
    bq_d = nc.dram_tensor("bq", [128, DC], F32, kind="ExternalInput").ap()
    bk_d = nc.dram_tensor("bk", [128, DC], F32, kind="ExternalInput").ap()
    bo_d = nc.dram_tensor("bo", [128, DC], F32, kind="ExternalInput").ap()
    beta_d = nc.dram_tensor("beta", [1, H], F32, kind="ExternalInput").ap()
    # output stored transposed [D, T]; host un-transposes
    out_d = nc.dram_tensor("out", [D, T], F32, kind="ExternalOutput").ap()

    with tile.TileContext(nc) as tc, ExitStack() as ctx:
        const = ctx.enter_context(tc.tile_pool(name="const", bufs=1))
        kgkp = ctx.enter_context(tc.tile_pool(name="kgkp", bufs=4))
        scr = ctx.enter_context(tc.tile_pool(name="scr", bufs=3))
        wpool = ctx.enter_context(tc.tile_pool(name="w", bufs=18))
        big = ctx.enter_context(tc.tile_pool(name="big", bufs=12))
        hstw_p = ctx.enter_context(tc.tile_pool(name="hstw", bufs=6))
        kt_p = ctx.enter_context(tc.tile_pool(name="ktp", bufs=6))
        v_p = ctx.enter_context(tc.tile_pool(name="vp", bufs=4))
        kgt_p = ctx.enter_context(tc.tile_pool(name="kgtp", bufs=6))
        e_p = ctx.enter_context(tc.tile_pool(name="ep", bufs=8))
        r_p = ctx.enter_context(tc.tile_pool(name="rp", bufs=4))
        fin_p = ctx.enter_context(tc.tile_pool(name="finp", bufs=6))
        sm_p = ctx.enter_context(tc.tile_pool(name="smp", bufs=4))
        ps = ctx.enter_context(tc.tile_pool(name="ps", bufs=2, space="PSUM"))

        ones_bf = const.tile([128, 64], BF16, tag="ones_bf")
        nc.vector.memset(ones_bf[:], 1.0)

        # ---- DMA issue order tracks consumption order ----
        # kg_key first: phase 0 (cosine bias -> exp factor) gates the
        # scaled v-projection copies
        kgk_tiles = []
        for c in range(KC):
            kk = kgkp.tile([128, D], BF16, tag="kgk", name="kgk")
            nc.sync.dma_start(kk[:, 0:384], kgk_d[c * 128:(c + 1) * 128, 0:384])
            nc.sync.dma_start(kk[:, 384:768], kgk_d[c * 128:(c + 1) * 128, 384:768])
            kgk_tiles.append(kk)

        # kg_value.T straight into SBUF via the DMA xbar
        kgt = [kgt_p.tile([128, K], BF16, tag="kgt", name="kgt") for _ in range(DC)]
        for c in range(DC):
            nc.sync.dma_start_transpose(
                kgt[c][:], kgv_d[:, c * 128:(c + 1) * 128])

        wk_sb = []
        wv_sb = []
        wq_sb = []
        for c in range(DC):
            wk = wpool.tile([128, D], BF16, tag="w")
            nc.sync.dma_start(wk[:], wkt_d[c * 128:(c + 1) * 128, :])
            wk_sb.append(wk)
        for c in range(DC):
            wv = wpool.tile([128, D], BF16, tag="w")
            nc.sync.dma_start(wv[:], wvt_d[c * 128:(c + 1) * 128, :])
            wv_sb.append(wv)

        pl = const.tile([1, D], F32, tag="pl")
        nc.sync.dma_start(pl[:], pl_d)
        bt = const.tile([1, H], F32, tag="bt")
        nc.sync.dma_start(bt[:], beta_d)
        bq_sb = const.tile([128, DC], F32, tag="bq_sb")
        nc.sync.dma_start(bq_sb[:], bq_d)
        bk_sb = const.tile([128, DC], F32, tag="bk_sb")
        nc.sync.dma_start(bk_sb[:], bk_d)
        bo_sb = const.tile([128, DC], F32, tag="bo_sb")
        nc.sync.dma_start(bo_sb[:], bo_d)

        # hs.T window 0, then wq, then the rest of hs.T
        hstw = [hstw_p.tile([128, T], BF16, tag="hstw", name="hstw")
                for _ in range(DC)]
        for c in range(DC):
            nc.sync.dma_start_transpose(
                hstw[c][:, 0:TW], hs_d[0:TW, c * 128:(c + 1) * 128])
        for c in range(DC):
            wq = wpool.tile([128, D], BF16, tag="w")
            nc.sync.dma_start(wq[:], wqt_d[c * 128:(c + 1) * 128, :])
            wq_sb.append(wq)
        for tc4 in range(1, NTW):
            tws = slice(tc4 * TW, (tc4 + 1) * TW)
            for c in range(DC):
                nc.sync.dma_start_transpose(
                    hstw[c][:, tws], hs_d[tws, c * 128:(c + 1) * 128])
        wo_sb = []
        for c in range(DC):
            wo = wpool.tile([128, D], BF16, tag="w")
            nc.sync.dma_start(wo[:], wot_d[c * 128:(c + 1) * 128, :])
            wo_sb.append(wo)

        beta_bc = const.tile([128, H], F32, tag="beta_bc")
        nc.gpsimd.partition_broadcast(beta_bc[:], bt[:], channels=128)
        pl_bc = const.tile([128, D], F32, tag="pl_bc")
        nc.gpsimd.partition_broadcast(pl_bc[:], pl[:], channels=128)

        # ---------------- phase 0: cosine-sim bias -> exp factor ----------------
        bias_all = const.tile([128, KC * H], F32, tag="bias_all")
        ebv = const.tile([128, KC * H], F32, tag="ebv")
        ebrep = [const.tile([128, H * 64], BF16, tag=f"ebrep{c}", name="ebrep")
                 for c in range(KC)]

        pl_sq = scr.tile([128, D], F32, tag="scr", name="pl_sq")
        pnorm = sm_p.tile([128, 1], F32, tag="pnorm")
        nc.scalar.activation(pl_sq[:], pl_bc[:], Act.Square, accum_out=pnorm[:])
        nc.scalar.activation(pnorm[:], pnorm[:], Act.Sqrt)
        nc.vector.tensor_scalar_max(pnorm[:], pnorm[:], EPS)
        rp_vec = const.tile([128, 1], F32, tag="rp_vec")
        nc.vector.reciprocal(rp_vec[:], pnorm[:])

        for c in range(KC):
            kk = kgk_tiles[c]
            sq = scr.tile([128, D], F32, tag="scr")
            nrm = sm_p.tile([128, 1], F32, tag="nrm")
            nc.scalar.activation(sq[:], kk[:], Act.Square, accum_out=nrm[:])
            nc.scalar.activation(nrm[:], nrm[:], Act.Sqrt)
            nc.vector.tensor_scalar_max(nrm[:], nrm[:], EPS)
            rn = sm_p.tile([128, 1], F32, tag="rn")
            nc.vector.reciprocal(rn[:], nrm[:])
            sq2 = scr.tile([128, D], F32, tag="scr")
            dot = sm_p.tile([128, 1], F32, tag="dot")
            nc.vector.scalar_tensor_tensor(
                out=sq2[:], in0=kk[:], scalar=1.0, in1=pl_bc[:],
                op0=Alu.mult, op1=Alu.mult, accum_out=dot[:])
            nc.vector.tensor_mul(dot[:], dot[:], rn[:])
            nc.vector.tensor_mul(dot[:], dot[:], rp_vec[:])
            nc.vector.tensor_scalar_mul(
                bias_all[:, c * H:(c + 1) * H], beta_bc[:], dot[:])

        nc.scalar.activation(ebv[:], bias_all[:], Act.Exp)
        for c in range(KC):
            for h in range(H):
                col = c * H + h
                nc.scalar.activation(
                    ebrep[c][:, h * 64:(h + 1) * 64],
                    ones_bf[:, 0:64], Act.Copy, scale=ebv[:, col:col + 1])

        # ---------------- phase 1a: k.T and v' ----------------
        kt = [kt_p.tile([128, K], BF16, tag="kt", name="kt") for _ in range(DC)]
        for m in range(DC):
            pk = ps.tile([128, K], F32, tag="od", bufs=4)
            for c in range(DC):
                nc.tensor.matmul(
                    pk[:], wk_sb[c][:, m * 128:(m + 1) * 128], kgt[c][:],
                    start=(c == 0), stop=(c == DC - 1))
            nc.vector.tensor_scalar_add(kt[m][:], pk[:], bk_sb[:, m:m + 1])

        # v' = v * exp(b): the scale is applied during the psum->sbuf
        # copy on the scalar engine (per-partition scale AP)
        v_sb = [v_p.tile([128, D], BF16, tag="v", name="vsb")
                for _ in range(KC)]
        for kc in range(KC):
            for n in range(2):
                pv = ps.tile([128, 384], F32, tag="od", bufs=4)
                for c in range(DC):
                    nc.tensor.matmul(
                        pv[:], kgt[c][:, kc * 128:(kc + 1) * 128],
                        wv_sb[c][:, n * 384:(n + 1) * 384],
                        start=(c == 0), stop=(c == DC - 1))
                for hh in range(6):
                    h = n * 6 + hh
                    col = kc * H + h
                    nc.scalar.activation(
                        v_sb[kc][:, h * 64:(h + 1) * 64],
                        pv[:, hh * 64:(hh + 1) * 64], Act.Copy,
                        scale=ebv[:, col:col + 1])

        # ---------------- phase 1b: q.T ----------------
        qt = [big.tile([128, T], BF16, tag="big", name="qt") for _ in range(DC)]
        for tc4 in range(NTW):
            tws = slice(tc4 * TW, (tc4 + 1) * TW)
            for m in range(DC):
                pq = ps.tile([128, TW], F32, tag="s", bufs=2)
                for c in range(DC):
                    nc.tensor.matmul(
                        pq[:], wq_sb[c][:, m * 128:(m + 1) * 128],
                        hstw[c][:, tws],
                        start=(c == 0), stop=(c == DC - 1))
                nc.vector.tensor_scalar_add(
                    qt[m][:, tws], pq[:], bq_sb[:, m:m + 1])

        # ------- attention + final projection, per t-window of 512 -------
        ot = [big.tile([128, T], BF16, tag="big", name="ot") for _ in range(NPAIR)]
        for tb in range(NTW):
            tws = slice(tb * TW, (tb + 1) * TW)
            for j in range(NPAIR):
                he = 2 * j
                ho = 2 * j + 1
                # scores + batched exp: per kc one [128,1024] psum tile
                # spanning two banks = [even(512) | odd(512)]; one
                # bias-free exp covers both heads
                e_j = []
                for kc in range(KC):
                    pse = ps.tile([128, 2 * TW], F32, tag="s", bufs=2)
                    nc.tensor.matmul(
                        pse[:, 0:TW],
                        kt[j][0:64, kc * 128:(kc + 1) * 128],
                        qt[j][0:64, tws], start=True, stop=True)
                    nc.tensor.matmul(
                        pse[:, TW:2 * TW],
                        kt[j][64:128, kc * 128:(kc + 1) * 128],
                        qt[j][64:128, tws], start=True, stop=True)
                    ee = e_p.tile([128, 2 * TW], BF16, tag="e")
                    nc.scalar.activation(ee[:], pse[:], Act.Exp)
                    e_j.append(ee)

                # AV + denominator: po = [evenAV ; oddAV],
                # pd = [oddDen ; evenDen]; each v-matmul pairs with an
                # ebrep-matmul on the opposite col-group -> concurrent
                po = ps.tile([128, TW], F32, tag="od", bufs=4, name="po")
                pd = ps.tile([128, TW], F32, tag="od", bufs=4, name="pd")
                for kc in range(KC):
                    st = (kc == 0)
                    sp = (kc == KC - 1)
                    nc.tensor.matmul(
                        po[0:64, :],
                        v_sb[kc][:, he * HD:(he + 1) * HD],
                        e_j[kc][:, 0:TW], start=st, stop=sp)
                    nc.tensor.matmul(
                        pd[64:128, :],
                        ebrep[kc][:, he * HD:(he + 1) * HD],
                        e_j[kc][:, 0:TW], start=st, stop=sp,
                        tile_position=(0, 64))
                for kc in range(KC):
                    st = (kc == 0)
                    sp = (kc == KC - 1)
                    nc.tensor.matmul(
                        po[64:128, :],
                        v_sb[kc][:, ho * HD:(ho + 1) * HD],
                        e_j[kc][:, TW:2 * TW], start=st, stop=sp)
                    nc.tensor.matmul(
                        pd[0:64, :],
                        ebrep[kc][:, ho * HD:(ho + 1) * HD],
                        e_j[kc][:, TW:2 * TW], start=st, stop=sp,
                        tile_position=(0, 0))

                rall = r_p.tile([128, TW], F32, tag="rall", name="rall")
                nc.vector.reciprocal_approx_fast(rall[:], pd[:])
                nc.vector.tensor_mul(
                    ot[j][0:64, tws], po[0:64, :], rall[64:128, :])
                nc.vector.tensor_mul(
                    ot[j][64:128, tws], po[64:128, :], rall[0:64, :])

            # final projection for this t-window, transposed: out.T[dout,t]
            for m in range(DC):
                pf = ps.tile([128, TW], F32, tag="od", bufs=4)
                for c in range(DC):
                    nc.tensor.matmul(
                        pf[:], wo_sb[c][:, m * 128:(m + 1) * 128],
                        ot[c][:, tws],
                        start=(c == 0), stop=(c == DC - 1))
                fin = fin_p.tile([128, TW], F32, tag="fin")
                nc.vector.tensor_scalar_add(
                    fin[:], pf[:], bo_sb[:, m:m + 1])
                nc.sync.dma_start(
                    out_d[m * 128:(m + 1) * 128, tws], fin[:])

    nc.compile()
    return nc


def _get_program():
    if "nc" not in _CACHE:
        _CACHE["nc"] = _build_program()
    return _CACHE["nc"]


def _host_prep(inputs):
    import ml_dtypes
    bf16 = ml_dtypes.bfloat16

    f32 = lambda x: np.ascontiguousarray(np.asarray(x, dtype=np.float32))
    Wq, Wk, Wv, Wo = (f32(inputs[k]) for k in ("Wq", "Wk", "Wv", "Wo"))
    bq, bk, bv, bo = (f32(inputs[k]) for k in ("bq", "bk", "bv", "bo"))
    beta = f32(inputs["beta"])

    shared = {
        "wqt": np.ascontiguousarray((Wq.T * SCALE).astype(bf16)),
        "wkt": np.ascontiguousarray(Wk.T.astype(bf16)),
        "wvt": np.ascontiguousarray(Wv.T.astype(bf16)),
        "wot": np.ascontiguousarray(Wo.T.astype(bf16)),
        "bq": np.ascontiguousarray((bq * SCALE).reshape(DC, 128).T),
        "bk": np.ascontiguousarray(bk.reshape(DC, 128).T),
        # bv folded through Wo (sum_k softmax == 1), bo absorbed:
        "bo": np.ascontiguousarray((bo + bv @ Wo.T).reshape(DC, 128).T),
        "beta": np.ascontiguousarray(beta.reshape(1, H)),
    }

    hs = np.asarray(inputs["hidden_states"], dtype=np.float32)
    kgk = np.asarray(inputs["kg_key"], dtype=np.float32)
    kgv = np.asarray(inputs["kg_value"], dtype=np.float32)
    pooled = f32(inputs["pooled_hidden_states"])

    in_maps = []
    for b in range(BS):
        m = dict(shared)
        m["hs"] = np.ascontiguousarray(hs[b].astype(bf16))
        m["kgk"] = np.ascontiguousarray(kgk[b].astype(bf16))
        m["kgv"] = np.ascontiguousarray(kgv[b].astype(bf16))
        m["pooled"] = np.ascontiguousarray(pooled[b].reshape(1, D))
        in_maps.append(m)
    return in_maps




def _install_ntff_hook():
    """Register the axon NTFF profile hook so trace=True yields exec_time_ns.

    Only used from our own test harness (TRACE=True); the default kernel()
    path never calls this.
    """
    try:
        from antenv.axon_hooks import get_axon_ntff_profile_hook  # noqa: F401
        return
    except ImportError:
        pass
    import contextlib
    import ctypes
    import types

    so_path = "/opt/axon/libaxon_pjrt.so"
    try:
        lib = ctypes.CDLL(so_path)
    except OSError:
        return
    if not hasattr(lib, "axon_start_nrt_profile"):
        return
    lib.axon_start_nrt_profile.argtypes = [
        ctypes.POINTER(ctypes.c_int64), ctypes.c_size_t]
    lib.axon_start_nrt_profile.restype = ctypes.c_int64
    lib.axon_stop_nrt_profile.argtypes = [ctypes.c_char_p]
    lib.axon_stop_nrt_profile.restype = ctypes.c_int64

    @contextlib.contextmanager
    def _hook(output_dir, device_ids):
        import jax
        jax.devices()
        if device_ids:
            ids = (ctypes.c_int64 * len(device_ids))(*device_ids)
            rc = lib.axon_start_nrt_profile(ids, len(device_ids))
        else:
            rc = lib.axon_start_nrt_profile(None, 0)
        if rc != 0:
            raise RuntimeError(f"axon_start_nrt_profile rc={rc}")
        try:
            yield
        finally:
            n = lib.axon_stop_nrt_profile(str(output_dir).encode())
            print(f"profile: {n} file(s) written to {output_dir}",
                  file=sys.stderr)

    mod = types.ModuleType("antenv.axon_hooks")
    mod.get_axon_ntff_profile_hook = lambda: _hook
    mod.set_axon_ntff_profile_hook = lambda h: None
    sys.modules["antenv.axon_hooks"] = mod


def kernel(**inputs):
    global LAST_EXEC_NS
    _ensure_path()
    from concourse import bass_utils

    if TRACE:
        _install_ntff_hook()
    nc = _get_program()
    in_maps = _host_prep(inputs)
    res = bass_utils.run_bass_kernel_spmd(
        nc, in_maps, core_ids=list(range(BS)), trace=TRACE)
    LAST_EXEC_NS = res.exec_time_ns
    # device output is out.T [D, T]; un-transpose per example
    out = np.stack([res.results[b]["out"].T for b in range(BS)], axis=0)
    return np.ascontiguousarray(out).astype(np.float32)
